# revision 50
# baseline (speedup 1.0000x reference)
"""Trainium2 Bass kernel for nn_CausalSelfAttention_90168543412719.

Sharding: head-parallel over the 32 attention heads (4 heads/core on 8
NeuronCores). Each core computes q/k/v projections for its heads from the
full x, runs causal + adapter-prefix + whisper cross attention for its
heads, then a per-batch AllToAll reshards y from head-sharded to
token-sharded and each core applies c_proj to its own 256 token rows.
Whisper K/V MLP is row-sharded across the 5 whisper cores.

All matmuls run in bf16 with fp32 PSUM accumulation. Host pre-slices /
pre-transposes / pre-casts every operand into the exact layout the PE
wants (x / qkv / whisper / cproj weights are host-pre-tiled so device DMAs
are contiguous slabs), so the device never transposes anything.

Phase order puts the qkv GEMM (pure PE work) first so the whisper-MLP
weight streams prefetch underneath it. Whisper keys are padded 1500->1536
so every kv loop runs 12 uniform 128-key tiles; the 36 tail keys are
killed with a per-partition bias of -30000 on the exp of the last tile.

Rope layout trick: the q/k head dims are permuted to [evens..., odds...]
(host permutes the corresponding weight columns), so rope becomes four
contiguous 64-partition block ops. Scores contract over the permuted dim
on both sides, so the permutation cancels; v / y stay in natural order.

Attention works in transposed score space: s_T[keys, q] = k_T.T @ q_T, so
probabilities come out in the exact [keys, q] layout the AV matmul wants
as rhs (no P transposes). Softmax denominators are column sums computed
on the PE with a ones vector; no max-shift is needed at these scales.

Performance structure (evidence from NTFF/perfetto profiles):
- Softmax denominators for a unit land on PSUM partitions {0,32,64,96}
  of ONE bank, so a single batched DVE reciprocal serves all branches;
  probability tiles are pairwise-summed on the DVE so only one
  denominator matmul runs per two key tiles.
- Filler PE work (whisper MLP, pv up-projection, pk assembly) is emitted
  BETWEEN a unit's denominator completion and its normalize matmuls, so
  the PE never waits on the DVE reciprocal chain.
- SBUF-only elementwise work (pk/pv adds, adapter normalize, whisper h
  scaling) runs on the otherwise-idle Pool engine; PSUM evacuations pick
  whichever of ACT/DVE is cold in that phase.
- The AllToAll payload is staged [dest, dim, head, tok] so the receive
  gather is 1KB-contiguous; c_proj weights stream through a 4-deep
  half-tile ring, each column block fetched ONCE and consumed by both
  token halves back-to-back (tail is PE-bound, zero steady-state gaps).
- kT / v / rope tables / A1-only whisper constants live in right-side
  stack pools that die after the causal phase, freeing SBUF for the
  c_proj ring; rms weights are folded into proj_down on the host.
"""

import os
import sys
from contextlib import ExitStack

import numpy as np
import ml_dtypes

for _p in ("/root/.axon_site/_ro/trn_rl_repo", "/opt/trn_rl_repo"):
    if os.path.isdir(_p) and _p not in sys.path:
        sys.path.append(_p)

import concourse.bass as bass
import concourse.mybir as mybir
import concourse.tile as tile
from concourse.bass_utils import run_bass_kernel_spmd

BF16 = mybir.dt.bfloat16
F32 = mybir.dt.float32
NBF = ml_dtypes.bfloat16
AF = mybir.ActivationFunctionType
ALU = mybir.AluOpType

B, T, C = 2, 1024, 4096
NH, HS = 32, 128
NCORES, HPC = 8, 4  # heads per core
A_LEN = 10
AT, AD, DD = 1500, 1280, 80  # audio_t, audio_d, down dim
AT2 = 1536  # whisper keys padded to 12*128
NWH, WHD = 20, 64  # whisper heads / head dim
EPS = 1e-5
BT = B * T  # 2048 global tokens, b-major
TT = 512  # token tile (matmul free dim)
NTT = BT // TT  # 4
TPC = BT // NCORES  # 256 tokens per core for c_proj
SCALE = 1.0 / float(np.sqrt(HS))
NEG = -30000.0  # additive mask value pre-scale; exp(NEG*SCALE) == 0 in f32
NKT = AT2 // 128  # 12 whisper key tiles per batch
KO = C // 128  # 32 contraction tiles over C
NOT = AD // 128  # 10 whisper tiles over AD

PERM = np.concatenate([np.arange(0, HS, 2), np.arange(1, HS, 2)])  # 128
PERM64 = np.concatenate([np.arange(0, WHD, 2), np.arange(1, WHD, 2)])  # 64

_PROG_CACHE = {}
_MAX_WAITS = 1


def _split_multi_waits(nc):
    """walrus here rejects >1 semaphore wait per instruction; hoist extras
    onto preceding NoOps on the same engine."""
    for f in nc.m.functions:
        for blk in f.blocks:
            insts = list(blk.instructions)
            new = []
            changed = False
            for inst in insts:
                si = inst.sync_info
                if si is not None and si.on_wait and len(si.on_wait) > _MAX_WAITS:
                    waits = list(si.on_wait)
                    keep = waits[-_MAX_WAITS:]
                    extra = waits[:-_MAX_WAITS]
                    for i in range(0, len(extra), _MAX_WAITS):
                        new.append(
                            mybir.InstNoOp(
                                name=f"{inst.name}.wsplit{i}",
                                engine=inst.engine,
                                debug=inst.debug,
                                sync_info=mybir.SyncInfo(
                                    on_wait=extra[i : i + _MAX_WAITS], on_update=[]
                                ),
                                bass_nofuse=True,
                            )
                        )
                    inst.sync_info = mybir.SyncInfo(
                        on_wait=keep, on_update=list(si.on_update)
                    )
                    changed = True
                new.append(inst)
            if changed:
                try:
                    blk.instructions[:] = new
                except TypeError:
                    blk.instructions = new


def build_program(gating_factor: float, proj_gating: float) -> bass.Bass:
    nc = bass.Bass()

    # ---------------- I/O (per-core data arrives via in_maps)
    xT = nc.dram_tensor("xT", [NTT, 128, KO, TT], BF16, kind="ExternalInput")
    wq = nc.dram_tensor("wq", [HPC, 128, KO, HS], BF16, kind="ExternalInput")
    wk = nc.dram_tensor("wk", [HPC, 128, KO, HS], BF16, kind="ExternalInput")
    wv = nc.dram_tensor("wv", [128, KO, HPC * HS], BF16, kind="ExternalInput")
    cosT = nc.dram_tensor("cosT", [HS // 2, T], F32, kind="ExternalInput")
    sinT = nc.dram_tensor("sinT", [HS // 2, T], F32, kind="ExternalInput")
    masks = nc.dram_tensor("masks", [128, 128], F32, kind="ExternalInput")
    akT = nc.dram_tensor("akT", [HPC, HS, A_LEN], BF16, kind="ExternalInput")
    avd = nc.dram_tensor("avd", [HPC, A_LEN, HS], BF16, kind="ExternalInput")
    aTd = nc.dram_tensor("aT", [AD, B * 300], BF16, kind="ExternalInput")
    wkey = nc.dram_tensor("wkey", [NOT, 128, NOT, 128], BF16, kind="ExternalInput")
    wval = nc.dram_tensor("wval", [NOT, 128, NOT, 128], BF16, kind="ExternalInput")
    vbias = nc.dram_tensor("vbias", [128, NOT], F32, kind="ExternalInput")
    rmsk = nc.dram_tensor("rmsk", [128, NOT], F32, kind="ExternalInput")
    rmsv = nc.dram_tensor("rmsv", [128, NOT], F32, kind="ExternalInput")
    pdown = nc.dram_tensor("pdown", [2, AD, DD], BF16, kind="ExternalInput")
    pupk = nc.dram_tensor("pupk", [DD, 20 * WHD], BF16, kind="ExternalInput")
    pupv = nc.dram_tensor("pupv", [DD, AD], BF16, kind="ExternalInput")
    padkT = nc.dram_tensor("padkT", [B, HS, AT2], BF16, kind="ExternalInput")
    padvT = nc.dram_tensor("padvT", [B, 128, NKT, WHD], BF16, kind="ExternalInput")
    padv0 = nc.dram_tensor("padv0", [B, 128, NKT, WHD], BF16, kind="ExternalInput")
    cproj = nc.dram_tensor("cproj", [C // TT, KO, 128, TT], BF16, kind="ExternalInput")
    out = nc.dram_tensor("out", [TPC, C], F32, kind="ExternalOutput")

    gf = float(gating_factor)
    pg = float(proj_gating)

    with tile.TileContext(nc) as tc, ExitStack() as ctx:
        dram = ctx.enter_context(tc.tile_pool(name="dram", bufs=1, space="DRAM"))
        const = ctx.enter_context(tc.tile_pool(name="const", bufs=1))
        persist = ctx.enter_context(tc.tile_pool(name="persist", bufs=1))

        # Collective bounce (split per batch) + whisper pv staging in DRAM
        a2a0_in = dram.tile([NCORES, HS, HPC, 128], BF16)
        a2a0_out = dram.tile([NCORES, HS, HPC, 128], BF16)
        a2a1_in = dram.tile([NCORES, HS, HPC, 128], BF16)
        a2a1_out = dram.tile([NCORES, HS, HPC, 128], BF16)
        a2a_ins = [a2a0_in, a2a1_in]
        a2a_outs = [a2a0_out, a2a1_out]
        pv_d = dram.tile([B, HPC, AT2 * WHD], BF16)  # per-(b,head) flat pv rows

        ones_bf = const.tile([128, 1], BF16)
        nc.gpsimd.memset(ones_bf[:], 1.0)
        ones_row = const.tile([1, 128], BF16)
        nc.gpsimd.memset(ones_row[:], 1.0)
        ones128 = const.tile([128, 128], BF16)
        nc.gpsimd.memset(ones128[:], 1.0)
        ones128f = const.tile([128, 128], F32)
        nc.gpsimd.memset(ones128f[:], 1.0)
        eps_sb = const.tile([1, 1], F32)
        nc.gpsimd.memset(eps_sb[:], EPS)
        tailb = const.tile([128, 1], F32)  # kill keys 1500:1536 in last tile
        nc.gpsimd.memset(tailb[:], NEG)
        nc.gpsimd.memset(tailb[0 : AT - 11 * 128, :], 0.0)
        zrow = const.tile([AT2 - AT, WHD], BF16)
        nc.gpsimd.memset(zrow[:], 0.0)

        # SBUF state persisting through attention (freed before phase P)
        mid = ctx.enter_context(ExitStack())
        midp = mid.enter_context(tc.tile_pool(name="midp", bufs=1))
        qT_sb = midp.tile([128, HPC, NTT, TT], BF16)  # roped q, permuted dims
        # right-side pools die after A1, freeing SBUF for the c_proj
        # weight ring before it opens (stack allocator is per-side LIFO)
        rts = ExitStack()
        kTp = rts.enter_context(tc.tile_pool(name="kTp", bufs=1, side="right"))
        kT_sb = kTp.tile([128, HPC, NTT, TT], BF16)  # roped k, permuted dims
        v_sb = kTp.tile([128, NTT, 4, HPC * HS], BF16)  # [tok128, tt, st, cols]
        cos_sb = kTp.tile([64, T], F32)
        sin_sb = kTp.tile([64, T], F32)
        mask_sb = kTp.tile([128, 128], F32)
        akT_sb = const.tile([128, HPC, A_LEN], BF16)
        av_sb = const.tile([A_LEN, HPC, HS], BF16)
        dk_loc = persist.tile([DD, B * 300], BF16)  # whisper down-proj, own rows
        dv_loc = persist.tile([DD, B * 300], BF16)

        # W1 constants (outside Q's pools so the DMAs overlap Q). The
        # A1-only set lives on the right side and dies with the fillers;
        # pupv_sb survives into A2-b0 (w2_unit(1, *)) so it stays in whc.
        whc = mid.enter_context(tc.tile_pool(name="whc", bufs=1))
        whcA = rts.enter_context(tc.tile_pool(name="whcA", bufs=1, side="right"))
        aT_sb = whcA.tile([128, NOT, B * 300], BF16)
        pdown_sb = whcA.tile([128, 2, NOT, DD], BF16)
        vb_sb = whcA.tile([128, NOT], F32)
        pupv_sb = whc.tile([DD, AD], BF16)

        def deferred_const_dmas():
            # issued after the first Q tiles so the critical first matmul
            # chain is not starved by prefetch traffic; rope needs cos/sin
            # early, the rest is small
            nc.sync.dma_start(cos_sb[:], cosT[:])
            nc.sync.dma_start(sin_sb[:], sinT[:])
            nc.sync.dma_start(mask_sb[:], masks[:])
            nc.sync.dma_start(vb_sb[:], vbias[:])

        def deferred_const_dmas2():
            # the bulky whisper/adapter constants wait until the second x
            # tile is in flight so they don't delay Q's weight prefetches
            nc.sync.dma_start(akT_sb[:], akT[:].rearrange("h p a -> p h a"))
            nc.sync.dma_start(av_sb[:], avd[:].rearrange("h a d -> a h d"))
            nc.sync.dma_start(aT_sb[:], aTd[:].rearrange("(ko p) r -> p ko r", p=128))
            nc.sync.dma_start(
                pdown_sb[:], pdown[:].rearrange("v (ko p) n -> p v ko n", p=128)
            )
            nc.sync.dma_start(pupv_sb[:], pupv[:])

        # =============== Phase Q: qkv projection + rope. Token tiles
        # are processed in PAIRS so each q/k weight LDWEIGHTS serves two
        # matmuls (the exposed weight-load gap halves). Per pair the
        # order is v(first tile) -> q/k(both) -> v(second tile), so the
        # second tile's x stream and the next pair's loads always hide
        # under ~35us of compute.
        with (
            tc.tile_pool(name="qx", bufs=2) as qx,
            tc.tile_pool(name="qw", bufs=2) as qw,
            tc.tile_pool(name="qwv", bufs=1) as qwv,
            tc.tile_pool(name="qpk", bufs=4, space="PSUM") as qpk,
            tc.tile_pool(name="qpv", bufs=4, space="PSUM") as qpv,
            tc.tile_pool(name="qt", bufs=2) as qtp,
        ):
            wv_w = qwv.tile([128, KO, HPC * HS], BF16)

            def rope(ps, dst, hl, tt):
                co = (tt % 2) * TT  # rope position offset within batch
                ev, od = ps[0:64, :], ps[64:128, :]
                cs = cos_sb[:, co : co + TT]
                sn = sin_sb[:, co : co + TT]
                t1 = qtp.tile([64, TT], F32, tag="r1")
                t2 = qtp.tile([64, TT], F32, tag="r2")
                nc.vector.tensor_tensor(t1[:], ev, cs, ALU.mult)
                nc.vector.tensor_tensor(t2[:], od, sn, ALU.mult)
                nc.vector.tensor_sub(dst[0:64, hl, tt, :], t1[:], t2[:])
                nc.vector.tensor_tensor(t1[:], od, cs, ALU.mult)
                nc.vector.tensor_tensor(t2[:], ev, sn, ALU.mult)
                nc.vector.tensor_add(dst[64:128, hl, tt, :], t1[:], t2[:])

            def v_proj(x_t, tt):
                # ko-outer with 4 st banks live so consumption tracks the
                # chunked x / wv streams ko-progressively
                pss = []
                for _i in range(4):
                    v_ps = qpv.tile([128, HPC * HS], F32, tag="v_ps")
                    pss.append(v_ps)
                for ko in range(KO):
                    for st in range(4):
                        nc.tensor.matmul(
                            pss[st][:],
                            x_t[:, ko, st * 128 : (st + 1) * 128],
                            wv_w[:, ko, :],
                            start=(ko == 0), stop=(ko == KO - 1),
                        )
                for st in range(4):
                    nc.scalar.copy(v_sb[:, tt, st, :], pss[st][:])

            for pair in range(NTT // 2):
                x_ts = []
                for j in range(2):
                    x_t = qx.tile([128, KO, TT], BF16, tag="x_t")
                    x_ts.append(x_t)
                # chunked, interleaved loads: first-tile and wv chunks
                # lead so v(first) starts immediately; second-tile chunks
                # stream underneath it
                for k0 in range(0, KO, 8):
                    nc.sync.dma_start(
                        x_ts[0][:, k0 : k0 + 8, :], xT[2 * pair, :, k0 : k0 + 8, :]
                    )
                    if pair == 0:
                        nc.sync.dma_start(
                            wv_w[:, k0 : k0 + 8, :], wv[:, k0 : k0 + 8, :]
                        )
                for k0 in range(0, KO, 8):
                    nc.sync.dma_start(
                        x_ts[1][:, k0 : k0 + 8, :], xT[2 * pair + 1, :, k0 : k0 + 8, :]
                    )
                if pair == 0:
                    deferred_const_dmas()
                v_proj(x_ts[0], 2 * pair)
                for ph in range(2):  # 0: q, 1: k
                    wsrc = wq if ph == 0 else wk
                    dst = qT_sb if ph == 0 else kT_sb
                    for hl in range(HPC):
                        w_t = qw.tile([128, KO, HS], BF16, tag="w_t")
                        nc.sync.dma_start(w_t[:], wsrc[hl])
                        if pair == 0 and ph == 1 and hl == 0:
                            deferred_const_dmas2()
                        # one LDWEIGHTS per ko serves both token tiles
                        pss = []
                        for _i in range(2):
                            qk_ps = qpk.tile([128, TT], F32, tag="qk_ps")
                            pss.append(qk_ps)
                        for ko in range(KO):
                            for j in range(2):
                                nc.tensor.matmul(
                                    pss[j][:], w_t[:, ko, :], x_ts[j][:, ko, :],
                                    start=(ko == 0), stop=(ko == KO - 1),
                                )
                        for j in range(2):
                            rope(pss[j], dst, hl, 2 * pair + j)
                v_proj(x_ts[1], 2 * pair + 1)

        # causal+adapter partial y, held until the whisper branch adds in.
        # Allocated after Q's pools close so it reuses their SBUF space.
        ostp = mid.enter_context(tc.tile_pool(name="ostp", bufs=1))
        o_store = ostp.tile([128, B * HPC * 2, TT], BF16)

        # =============== Phases A1+A2+P: attention + c_proj. A1 does
        # causal+adapter into o_store, with whisper-MLP (W1), pv
        # up-projection (W2) and the b=0 pk assembly interleaved as filler
        # work. A2 does whisper cross-attention, with b=1 prep interleaved
        # into b=0's slots and c_proj batch-0 chunks woven into b=1's
        # slots; each batch's AllToAll launches as soon as it is staged.
        # Softmax denominators for a unit land on PSUM partitions
        # {0,32,64,96} of one bank so ONE reciprocal + ONE cast serve all
        # branches, and filler PE work is emitted between the denominator
        # completion and the normalize matmuls so the PE never waits on
        # the DVE reciprocal chain.
        with (
            tc.tile_pool(name="w2", bufs=2) as w2,
            tc.tile_pool(name="ap", bufs=2) as ap,
            tc.tile_pool(name="apk", bufs=2) as apk,
            tc.tile_pool(name="apv", bufs=2) as apv,
            tc.tile_pool(name="pt2", bufs=6) as pt2,
            tc.tile_pool(name="ascp", bufs=2, space="PSUM") as ascp,
            tc.tile_pool(name="ayp", bufs=2, space="PSUM") as ayp,
            tc.tile_pool(name="adp", bufs=2, space="PSUM") as adp,
        ):
            # W1-only pools live in a nested scope freed after the A1
            # fillers drain, returning 2 PSUM banks + ~25KB/part of SBUF.
            w1s = ExitStack()
            wh = w1s.enter_context(tc.tile_pool(name="wh", bufs=1))
            whs = w1s.enter_context(tc.tile_pool(name="whs", bufs=2))
            whp_h = w1s.enter_context(tc.tile_pool(name="whp_h", bufs=1, space="PSUM"))
            whp_s = w1s.enter_context(tc.tile_pool(name="whp_s", bufs=1, space="PSUM"))
            whp_m = whp_h
            pupk_sb = apk.tile([DD, 20, WHD], BF16, tag="pupk")
            nc.sync.dma_start(pupk_sb[:], pupk[:].rearrange("d (u i) -> d u i", i=WHD))

            w1_state = {}
            pk4_t = {}

            def prefetch_w(kv, ot):
                w_t = whs.tile([128, NOT, 128], BF16, tag="wh_w")
                w_dram = wkey if kv == 0 else wval
                nc.sync.dma_start(w_t[:], w_dram[ot])
                w1_state["w_next"] = w_t

            def h_unit(kv, ot):
                w_t = w1_state["w_next"]
                if ot + 1 < NOT:
                    prefetch_w(kv, ot + 1)
                elif kv == 0:
                    prefetch_w(1, 0)
                if ot == 0:
                    h_cur = wh.tile([128, NOT, B * 300], BF16, tag="h_sb")
                    ssq_cur = whp_s.tile([33, 300], F32, tag="ssq")
                    w1_state["h"] = h_cur
                    w1_state["ssq"] = ssq_cur
                h_sb = w1_state["h"]
                ssq = w1_state["ssq"]
                for b2 in range(2):
                    c0 = 300 * b2
                    hp_t = whp_h.tile([128, TT], F32, tag="m")
                    hp = hp_t[:, 0:300]
                    for kt in range(NOT):
                        nc.tensor.matmul(
                            hp,
                            w_t[:, kt, :],
                            aT_sb[:, kt, c0 : c0 + 300],
                            start=(kt == 0), stop=(kt == NOT - 1),
                        )
                    if kv == 1:
                        nc.scalar.activation(
                            h_sb[:, ot, c0 : c0 + 300], hp, AF.Identity,
                            bias=vb_sb[:, ot : ot + 1],
                        )
                    else:
                        nc.scalar.copy(h_sb[:, ot, c0 : c0 + 300], hp)
                    hsq = wh.tile([128, 300], BF16, tag="hsq")
                    nc.scalar.activation(hsq[:], h_sb[:, ot, c0 : c0 + 300], AF.Square)
                    sr = 32 * b2
                    nc.tensor.matmul(
                        ssq[sr : sr + 1, :], ones_bf[:], hsq[:],
                        start=(ot == 0), stop=(ot == NOT - 1),
                        skip_group_check=True,
                    )

            def w1_tail(kv):
                d_dst = dk_loc if kv == 0 else dv_loc
                h_sb = w1_state["h"]
                ssq = w1_state["ssq"]
                for b2 in range(2):
                    c0 = 300 * b2
                    sq_sb = wh.tile([1, 300], F32, tag="sq_sb")
                    sr = 32 * b2
                    nc.scalar.activation(
                        sq_sb[:], ssq[sr : sr + 1, :], AF.Sqrt,
                        bias=eps_sb[:], scale=1.0 / AD,
                    )
                    rr_sb = wh.tile([1, 300], F32, tag="rr_sb")
                    rr_bf = wh.tile([1, 300], BF16, tag="rr_bf")
                    nc.vector.reciprocal(rr_sb[:], sq_sb[:])
                    nc.scalar.copy(rr_bf[:], rr_sb[:])
                    m1 = whp_m.tile([128, TT], F32, tag="m")
                    nc.tensor.matmul(
                        m1[:, 0:300], ones_row[:], rr_bf[:], start=True, stop=True
                    )
                    rrb = wh.tile([128, 300], F32, tag="rrb")
                    nc.scalar.copy(rrb[:], m1[:, 0:300])
                    hn_sb = wh.tile([128, NOT, 300], BF16, tag="hn_sb")
                    for ot in range(NOT):
                        # rms weights are folded into pdown on the host, so
                        # this is a plain multiply the idle Pool engine can do
                        nc.gpsimd.tensor_tensor(
                            hn_sb[:, ot, :], h_sb[:, ot, c0 : c0 + 300],
                            rrb[:], ALU.mult,
                        )
                    m2 = whp_m.tile([128, TT], F32, tag="m")
                    for kt in range(NOT):
                        nc.tensor.matmul(
                            m2[0:DD, 0:300], pdown_sb[:, kv, kt, :], hn_sb[:, kt, :],
                            start=(kt == 0), stop=(kt == NOT - 1),
                        )
                    nc.scalar.activation(d_dst[:, c0 : c0 + 300], m2[0:DD, 0:300], AF.Silu)

            def w2_unit(b, hl):
                # pv rows: head g keys [1500, 64] are wv rows [75g, 75g+75)
                # flat row-major; rows 1500:1536 (tail) zeroed.
                wvrow = w2.tile([128, AD], BF16, tag="wvrow")
                for ns in range(3):
                    n0 = ns * 512
                    nsz = min(512, AD - n0)
                    ps = ascp.tile([128, TT], F32, tag="sc")
                    nc.tensor.matmul(
                        ps[0:75, :nsz],
                        dv_loc[:, b * 300 + 75 * hl : b * 300 + 75 * (hl + 1)],
                        pupv_sb[:, n0 : n0 + nsz],
                        start=True, stop=True,
                    )
                    nc.scalar.copy(wvrow[0:75, n0 : n0 + nsz], ps[0:75, :nsz])
                nc.sync.dma_start(
                    pv_d[b, hl, : AT * WHD].rearrange("(r d) -> r d", r=75),
                    wvrow[0:75, :],
                )
                nc.sync.dma_start(
                    pv_d[b, hl, AT * WHD :].rearrange("(r d) -> r d", r=AT2 - AT),
                    zrow[:],
                )

            def pk4_start(b):
                # pk for all 4 heads: [128 dims, hl, 1536 keys].
                # pk[i, hl, 20*jr+u] = pad[i, key] (whisper rows host-zeroed)
                #                    + pupk[:, u, i] . dk[:, 75*hl+jr]
                pk4 = apk.tile([128, HPC, AT2], BF16, tag="pk4")
                pk4_t[b] = pk4
                for hl in range(HPC):
                    nc.sync.dma_start(pk4[:, hl, :], padkT[b, :, :])

            def pk4_chunk(b, u0):
                # PSUM is evacuated by the scalar engine and the adds run
                # on the idle gpsimd engine so they neither load the DVE
                # (whose queue backs up behind the unit tails) nor
                # head-of-line-block the PE stream
                pk4 = pk4_t[b]
                pk4v = pk4[:, :, :AT].rearrange("p h (j u) -> p h j u", u=20)
                dk4 = dk_loc[:, b * 300 : (b + 1) * 300]
                for u in range(u0, u0 + 5):
                    # b=1 chunks run inside A2-b0 where the shared score
                    # tag's consumers lag on the busy ACT/DVE queues; a
                    # dedicated bank decouples them from the PE stream
                    pool = pkq if b == 1 else ascp
                    tag = "pk" if b == 1 else "sc"
                    pkp = pool.tile([128, TT], F32, tag=tag)
                    nc.tensor.matmul(
                        pkp[0:32, 0:300], pupk_sb[:, u, 0:32], dk4,
                        start=True, stop=True,
                    )
                    nc.tensor.matmul(
                        pkp[64:96, 0:300], pupk_sb[:, u, 32:64], dk4,
                        start=True, stop=True,
                    )
                    pk_sb = w2.tile([128, 300], F32, tag="pksb")
                    copy = nc.scalar.copy if b == 0 else nc.vector.tensor_copy
                    for half in range(2):
                        p0 = 64 * half
                        copy(pk_sb[p0 : p0 + 32, :], pkp[p0 : p0 + 32, 0:300])
                    for half in range(2):
                        p0 = 64 * half
                        nc.gpsimd.tensor_add(
                            pk4v[p0 : p0 + 32, :, :, u],
                            pk_sb[p0 : p0 + 32, :].rearrange(
                                "p (h j) -> p h j", h=HPC
                            ),
                            pk4v[p0 : p0 + 32, :, :, u],
                        )

            prefetch_w(0, 0)
            fillers = []
            for ot in range(NOT):
                fillers.append(lambda ot=ot: h_unit(0, ot))
            fillers.append(lambda: w1_tail(0))
            # pk assembly right after dk is ready so the gpsimd add chain
            # drains long before A2's first scores read pk4
            fillers.append(lambda: pk4_start(0))
            for u0 in range(0, 20, 5):
                fillers.append(lambda u0=u0: pk4_chunk(0, u0))
            for ot in range(NOT):
                fillers.append(lambda ot=ot: h_unit(1, ot))
            fillers.append(lambda: w1_tail(1))
            for hl in range(HPC):
                fillers.append(lambda hl=hl: w2_unit(0, hl))

            n_units = B * HPC
            fidx = 0

            # ---------------- A1: causal + adapter. Denominators land on
            # PSUM rows {0: causal qt0, 32: causal qt1, 64: adapter qt0,
            # 96: adapter qt1} of one bank; one batched reciprocal + cast
            # serves all four, with filler PE work emitted in between so
            # the normalize matmuls never wait on the DVE.
            for b in range(B):
                for hl in range(HPC):
                    dt = adp.tile([97, TT], F32, tag="den")
                    ys = {}
                    ya_sbs = {}
                    for qt in range(2):
                        qcol = qT_sb[:, hl, 2 * b + qt, :]  # [128, 512]
                        nkt = 4 * (qt + 1)
                        y_ps = ayp.tile([128, TT], F32, tag="y")
                        dr = 32 * qt
                        pts = {}
                        for kt in range(nkt):
                            sp = ascp.tile([128, TT], F32, tag="sc")
                            nc.tensor.matmul(
                                sp[:],
                                kT_sb[:, hl, 2 * b + kt // 4,
                                      (kt % 4) * 128 : (kt % 4) * 128 + 128],
                                qcol, start=True, stop=True,
                            )
                            pt = pt2.tile([128, TT], BF16, tag="pt2")
                            roff = kt * 128 - qt * TT
                            if roff >= 0:  # diagonal block
                                # columns < roff are fully masked: zero them
                                # in pt (off the critical path); add the
                                # shared [128,128] triangular mask on the
                                # boundary column block; exp the rest.
                                if roff > 0:
                                    nc.gpsimd.memset(pt[:, 0:roff], 0.0)
                                nc.vector.tensor_add(
                                    sp[:, roff : roff + 128],
                                    sp[:, roff : roff + 128],
                                    mask_sb[:],
                                )
                                nc.scalar.activation(
                                    pt[:, roff:TT], sp[:, roff:TT], AF.Exp, scale=SCALE
                                )
                            else:
                                nc.scalar.activation(pt[:], sp[:], AF.Exp, scale=SCALE)
                            pts[kt] = pt
                            if kt >= 1:
                                ptp = pts[kt - 1]
                                nc.tensor.matmul(
                                    y_ps[:],
                                    v_sb[:, 2 * b + (kt - 1) // 4, (kt - 1) % 4,
                                         hl * HS : (hl + 1) * HS],
                                    ptp[:],
                                    start=(kt - 1 == 0), stop=False,
                                )
                                if kt % 2 == 0:
                                    pa_, pb_ = pts.pop(kt - 2), pts.pop(kt - 1)
                                    psum2 = pt2.tile([128, TT], BF16, tag="ptsum")
                                    nc.vector.tensor_add(psum2[:], pa_[:], pb_[:])
                                    nc.tensor.matmul(
                                        dt[dr : dr + 1, :], ones_bf[:], psum2[:],
                                        start=(kt == 2), stop=False,
                                        skip_group_check=True,
                                    )
                        ptp = pts[nkt - 1]
                        nc.tensor.matmul(
                            y_ps[:],
                            v_sb[:, 2 * b + (nkt - 1) // 4, (nkt - 1) % 4,
                                 hl * HS : (hl + 1) * HS],
                            ptp[:],
                            start=False, stop=True,
                        )
                        pa_, pb_ = pts.pop(nkt - 2), pts.pop(nkt - 1)
                        psum2 = pt2.tile([128, TT], BF16, tag="ptsum")
                        nc.vector.tensor_add(psum2[:], pa_[:], pb_[:])
                        nc.tensor.matmul(
                            dt[dr : dr + 1, :], ones_bf[:], psum2[:],
                            start=False, stop=True, skip_group_check=True,
                        )
                        ys[qt] = y_ps
                        # ---- adapter prefix attention: PE work inline;
                        # ya is evacuated to SBUF so its PSUM bank frees
                        # and normalization can defer to the unit tail.
                        sa = ascp.tile([128, TT], F32, tag="sc")
                        nc.tensor.matmul(
                            sa[0:A_LEN, :], akT_sb[:, hl, :], qcol, start=True, stop=True
                        )
                        pa = ap.tile([A_LEN, TT], BF16, tag="pa")
                        nc.scalar.activation(pa[:], sa[0:A_LEN, :], AF.Exp, scale=SCALE)
                        nc.tensor.matmul(
                            dt[64 + dr : 64 + dr + 1, :], ones_bf[0:A_LEN, :], pa[:],
                            start=True, stop=True, skip_group_check=True,
                            tile_position=(0, 64 + dr),
                        )
                        ya = ascp.tile([128, TT], F32, tag="sc")
                        nc.tensor.matmul(ya[:], av_sb[:, hl, :], pa[:], start=True, stop=True)
                        ya_sb = ap.tile([128, TT], F32, tag="yasb")
                        nc.scalar.copy(ya_sb[:], ya[:])
                        ya_sbs[qt] = ya_sb

                    # one reciprocal for all four denominators; the bf16
                    # cast runs on the scalar engine (fp32 rep matmuls run
                    # LOW_HIGH at 2x cost, so bf16 reps win)
                    rc = ap.tile([97, TT], F32, tag="rc")
                    rc_bf = ap.tile([97, TT], BF16, tag="rcbf")
                    nc.vector.reciprocal(rc[:], dt[:])
                    nc.scalar.copy(rc_bf[:], rc[:])

                    # filler PE work hides the recip+cast latency
                    unit = b * HPC + hl
                    want = ((unit + 1) * len(fillers)) // n_units
                    mid_stop = min(max(fidx + 2, want - 1), len(fillers))
                    while fidx < mid_stop:
                        fillers[fidx]()
                        fidx += 1

                    for qt in range(2):
                        oi = (b * HPC + hl) * 2 + qt
                        dr = 32 * qt
                        rep2 = ascp.tile([128, TT], F32, tag="sc")
                        nc.tensor.matmul(
                            rep2[:], ones128[64 + dr : 64 + dr + 1, :],
                            rc_bf[64 + dr : 64 + dr + 1, :],
                            start=True, stop=True, tile_position=(64 + dr, 0),
                        )
                        rep_sb2 = ap.tile([128, TT], F32, tag="repsb")
                        nc.scalar.copy(rep_sb2[:], rep2[:])
                        tmp = ap.tile([128, TT], F32, tag="tmp")
                        nc.gpsimd.tensor_tensor(tmp[:], ya_sbs[qt][:], rep_sb2[:], ALU.mult)
                        rep = ascp.tile([128, TT], F32, tag="sc")
                        nc.tensor.matmul(
                            rep[:], ones128[dr : dr + 1, :], rc_bf[dr : dr + 1, :],
                            start=True, stop=True,
                        )
                        rep_sb = ap.tile([128, TT], F32, tag="repsb")
                        nc.scalar.copy(rep_sb[:], rep[:])
                        o_sb = ap.tile([128, TT], F32, tag="o_sb")
                        nc.vector.tensor_tensor(o_sb[:], ys[qt][:], rep_sb[:], ALU.mult)
                        nc.vector.scalar_tensor_tensor(
                            o_store[:, oi, :], tmp[:], gf, o_sb[:], ALU.mult, ALU.add
                        )
                    while fidx < want:
                        fillers[fidx]()
                        fidx += 1
            while fidx < len(fillers):
                fillers[fidx]()
                fidx += 1
            w1s.close()
            rts.close()  # kT + A1-only whisper constants die here
            pps = ExitStack()
            pwp = pps.enter_context(tc.tile_pool(name="pw", bufs=4))
            py = pps.enter_context(tc.tile_pool(name="py", bufs=1))
            po = pps.enter_context(tc.tile_pool(name="po", bufs=2))
            pp = pps.enter_context(tc.tile_pool(name="pp", bufs=1, space="PSUM"))
            pkq = pps.enter_context(tc.tile_pool(name="pkq", bufs=1, space="PSUM"))

            # ---------------- P-phase machinery (c_proj). Batch-0 chunks
            # weave into A2-b1 as fillers; the weight ring re-streams
            # cproj for the batch-1 chunks at the end.
            NP = C // TT
            yT = {}

            def yT_load(half):
                t = py.tile([128, KO, 128], BF16, tag=f"yT{half}")
                nc.sync.dma_start(
                    t[:].rearrange("p (i h) t -> p i h t", h=HPC),
                    a2a_outs[half][:].rearrange("i d h t -> d i h t"),
                )
                yT[half] = t

            wring = {}
            KH = KO // 2

            def w_fetch(n, gen=0):
                # two half-tiles per column block so the ring stays at
                # 2x16KB/part and halves pipeline under the chunk matmuls
                for h in range(2):
                    w_h = pwp.tile([128, KH, TT], BF16, tag="w_n")
                    nc.sync.dma_start(
                        w_h[:],
                        cproj[n, h * KH : (h + 1) * KH, :, :].rearrange(
                            "ko p t -> p ko t"
                        ),
                    )
                    wring[(gen, n, h)] = w_h

            def p_chunk(n, half, gen=0, pop=True):
                ps = pp.tile([128, TT], F32, tag="o_ps")
                for h in range(2):
                    key = (gen, n, h)
                    w_h = wring.pop(key) if pop else wring[key]
                    for k in range(KH):
                        ko = h * KH + k
                        nc.tensor.matmul(
                            ps[:],
                            yT[half][:, ko, :],
                            w_h[:, k, :],
                            start=(ko == 0), stop=(ko == KO - 1),
                        )
                o_t = po.tile([128, TT], F32, tag="o_t")
                nc.scalar.copy(o_t[:], ps[:])
                nc.sync.dma_start(
                    out[half * 128 : (half + 1) * 128, n * TT : (n + 1) * TT], o_t[:]
                )

            # ---------------- A2: whisper cross attention
            a2f = []
            a2f.append(lambda: pk4_start(1))
            for u0 in range(0, 20, 5):
                a2f.append(lambda u0=u0: pk4_chunk(1, u0))
            for hl in range(HPC):
                a2f.append(lambda hl=hl: w2_unit(1, hl))
            a2n = 0

            for b in range(B):
                pk4 = pk4_t[b]
                for hl in range(HPC):
                    # pv [keys, kt, 128d]: cols 0:64 whisper rows (+pad for
                    # non-whisper cores via padv0), cols 64:128 pad.
                    pv4 = apv.tile([128, NKT, HS], BF16, tag="pv")
                    nc.sync.dma_start(
                        pv4[:, :, 0:WHD],
                        pv_d[b, hl, :].rearrange("(kt p d) -> p kt d", p=128, d=WHD),
                    )
                    nc.sync.dma_start(pv4[:, :, WHD:HS], padvT[b, :, :, :])
                    p0t = apv.tile([128, NKT, WHD], BF16, tag="p0t")
                    nc.sync.dma_start(p0t[:], padv0[b, :, :, :])
                    nc.gpsimd.tensor_add(pv4[:, :, 0:WHD], p0t[:], pv4[:, :, 0:WHD])

                    dt = adp.tile([97, TT], F32, tag="den")
                    # both query tiles' kt-loops interleaved: two independent
                    # score->exp->den/AV streams keep the PE fed while the
                    # Activation engine works through the exps
                    yw0 = ayp.tile([128, TT], F32, tag="y")
                    yw1 = ayp.tile([128, TT], F32, tag="y")
                    yws = {0: yw0, 1: yw1}
                    qcols = [qT_sb[:, hl, 2 * b + qt, :] for qt in range(2)]
                    pws = {}
                    for kt in range(NKT):
                        k0 = kt * 128
                        for qt in range(2):
                            sw = ascp.tile([128, TT], F32, tag="sc")
                            nc.tensor.matmul(
                                sw[:], pk4[:, hl, k0 : k0 + 128], qcols[qt],
                                start=True, stop=True,
                            )
                            pw = pt2.tile([128, TT], BF16, tag="pt2")
                            if kt == NKT - 1:
                                nc.scalar.activation(
                                    pw[:], sw[:], AF.Exp, bias=tailb[:], scale=SCALE
                                )
                            else:
                                nc.scalar.activation(pw[:], sw[:], AF.Exp, scale=SCALE)
                            pws[(qt, kt)] = pw
                        if kt >= 1:
                            for qt in range(2):
                                pwp_ = pws[(qt, kt - 1)]
                                nc.tensor.matmul(
                                    yws[qt][:], pv4[:, kt - 1, :], pwp_[:],
                                    start=(kt - 1 == 0), stop=False,
                                )
                                if kt % 2 == 0:
                                    # sum the pt pair on the DVE; one den
                                    # matmul per TWO key tiles
                                    pa_, pb_ = pws.pop((qt, kt - 2)), pws.pop((qt, kt - 1))
                                    psum2 = pt2.tile([128, TT], BF16, tag="ptsum")
                                    nc.vector.tensor_add(psum2[:], pa_[:], pb_[:])
                                    nc.tensor.matmul(
                                        dt[32 * qt : 32 * qt + 1, :], ones_bf[:],
                                        psum2[:],
                                        start=(kt == 2), stop=False,
                                        skip_group_check=True,
                                    )
                    for qt in range(2):
                        pwp_ = pws[(qt, NKT - 1)]
                        nc.tensor.matmul(
                            yws[qt][:], pv4[:, NKT - 1, :], pwp_[:], start=False, stop=True
                        )
                        pa_, pb_ = pws.pop((qt, NKT - 2)), pws.pop((qt, NKT - 1))
                        psum2 = pt2.tile([128, TT], BF16, tag="ptsum")
                        nc.vector.tensor_add(psum2[:], pa_[:], pb_[:])
                        nc.tensor.matmul(
                            dt[32 * qt : 32 * qt + 1, :], ones_bf[:], psum2[:],
                            start=False, stop=True, skip_group_check=True,
                        )

                    # one reciprocal for both denominators
                    rw = ap.tile([97, TT], F32, tag="rc")
                    rw_bf = ap.tile([97, TT], BF16, tag="rcbf")
                    nc.vector.reciprocal(rw[0:33, :], dt[0:33, :])
                    nc.vector.tensor_copy(rw_bf[0:33, :], rw[0:33, :])

                    # filler PE work hides the recip+cast latency
                    if b == 0:
                        want = ((hl + 1) * len(a2f)) // HPC
                        mid_stop = min(max(a2n + 2, want - 1), len(a2f))
                        while a2n < mid_stop:
                            a2f[a2n]()
                            a2n += 1

                    for qt in range(2):
                        oi = (b * HPC + hl) * 2 + qt
                        dr = 32 * qt
                        rep = ascp.tile([128, TT], F32, tag="sc")
                        nc.tensor.matmul(
                            rep[:], ones128[dr : dr + 1, :], rw_bf[dr : dr + 1, :],
                            start=True, stop=True,
                        )
                        rep_sb = ap.tile([128, TT], F32, tag="repsb")
                        nc.vector.tensor_copy(rep_sb[:], rep[:])
                        tmp = ap.tile([128, TT], F32, tag="tmp")
                        nc.vector.tensor_tensor(tmp[:], yws[qt][:], rep_sb[:], ALU.mult)
                        yfin = ap.tile([128, TT], BF16, tag="yfin")
                        nc.vector.scalar_tensor_tensor(
                            yfin[:], tmp[:], pg, o_store[:, oi, :], ALU.mult, ALU.add
                        )
                        # stage into this batch's a2a bounce: j = tok/128
                        for c4 in range(4):
                            nc.sync.dma_start(
                                a2a_ins[b][qt * 4 + c4, :, hl, :],
                                yfin[:, c4 * 128 : (c4 + 1) * 128],
                            )
                    if b == 0:
                        while a2n < want:
                            a2f[a2n]()
                            a2n += 1
                # batch b fully staged: launch its AllToAll (overlaps the
                # next batch's attention / c_proj chunks)
                nc.gpsimd.collective_compute(
                    "AllToAll",
                    ALU.bypass,
                    replica_groups=[list(range(NCORES))],
                    ins=[a2a_ins[b][:].opt()],
                    outs=[a2a_outs[b][:].opt()],
                )
                if b == 0:
                    yT_load(0)
                    w_fetch(0)

            # ---------------- P tail: each cproj column block streams
            # ONCE; batch-1 chunks lag one block behind batch-0 so the
            # first two batch-0 chunks cover the second collective + yT1
            # latency, and each weight is fully consumed before its ring
            # slots are refetched.
            yT_load(1)
            w_fetch(1)
            for n in range(NP):
                p_chunk(n, 0, pop=False)
                p_chunk(n, 1)
                if n + 2 < NP:
                    w_fetch(n + 2)
            pps.close()

        mid.close()

    _split_multi_waits(nc)
    return nc


def prepare_inputs(inputs):
    """Host-side slicing / casting / transposition. Returns in_maps (one
    dict per core)."""
    f32 = np.float32
    x = np.asarray(inputs["x"], f32)
    audio = np.asarray(inputs["audio_features"], f32)
    rope_cos = np.asarray(inputs["rope_cos"], f32)
    rope_sin = np.asarray(inputs["rope_sin"], f32)
    pad_k = np.asarray(inputs["pad_base_k"], f32)
    pad_v = np.asarray(inputs["pad_base_v"], f32)
    c_attn = np.asarray(inputs["c_attn_w"], f32)
    c_proj = np.asarray(inputs["c_proj_w"], f32)
    adapter_wte = np.asarray(inputs["adapter_wte"], f32)
    rms_gate = np.asarray(inputs["rms_gate_w"], f32)
    rms_key = np.asarray(inputs["rms_key_w"], f32)
    rms_val = np.asarray(inputs["rms_value_w"], f32)
    p_down = np.asarray(inputs["proj_down"], f32)
    p_up = np.asarray(inputs["proj_up"], f32)
    wh_k = np.asarray(inputs["whisper_key_w"], f32)
    wh_v = np.asarray(inputs["whisper_value_w"], f32)
    wh_vb = np.asarray(inputs["whisper_value_b"], f32)

    assert np.array_equal(
        np.asarray(inputs["proj_q128"], f32), np.eye(HS, dtype=f32)
    ) and np.array_equal(
        np.asarray(inputs["proj_q32"], f32), np.eye(NH, dtype=f32)
    ), "general q-reprojection path not implemented"
    mask = np.asarray(inputs["mask"])
    assert mask.shape == (1, 1, T, T)
    assert np.array_equal(
        mask[0, 0], np.tril(np.ones((T, T), dtype=bool))
    ), "only causal mask supported"

    # pre-tiled [tt, p, ko, t]: per-partition-contiguous 32KB DMA slabs
    xT = np.ascontiguousarray(
        x.reshape(NTT, TT, KO, 128).transpose(0, 3, 2, 1).astype(NBF)
    )

    # adapter k/v on host (tiny)
    ms = np.mean(adapter_wte * adapter_wte, axis=-1, keepdims=True)
    prefix = adapter_wte / np.sqrt(ms + EPS) * rms_gate
    aqkv = prefix @ c_attn
    ak = aqkv[:, C : 2 * C].reshape(A_LEN, NH, HS)
    av = aqkv[:, 2 * C :].reshape(A_LEN, NH, HS)

    cosT = np.ascontiguousarray(rope_cos.T)
    sinT = np.ascontiguousarray(rope_sin.T)

    # shared [128,128] triangular mask for the diag-boundary column block
    kk = np.arange(128)[:, None]
    jj = np.arange(128)[None, :]
    masks = np.where(jj >= kk, 0.0, NEG).astype(f32)

    wkey_t = np.ascontiguousarray(
        wh_k.astype(NBF).reshape(NOT, 128, NOT, 128).transpose(2, 1, 0, 3)
    )
    wval_t = np.ascontiguousarray(
        wh_v.astype(NBF).reshape(NOT, 128, NOT, 128).transpose(2, 1, 0, 3)
    )
    vb_t = np.ascontiguousarray(wh_vb.reshape(NOT, 128).T)
    rmsk_t = np.ascontiguousarray(rms_key.reshape(NOT, 128).T)
    rmsv_t = np.ascontiguousarray(rms_val.reshape(NOT, 128).T)
    padkT_perm = np.ascontiguousarray(pad_k.transpose(0, 2, 1)[:, PERM, :])
    cproj_b = np.ascontiguousarray(
        c_proj.astype(NBF).reshape(KO, 128, C // TT, TT).transpose(2, 0, 1, 3)
    )
    aT_full = np.ascontiguousarray(audio.reshape(B * AT, AD).T)  # [1280, 3000]
    # pupk col (u, i) = proj_up[:, 64u + PERM64[i]]
    pupk_all = np.empty((DD, 20 * WHD), f32)
    for u in range(20):
        pupk_all[:, u * WHD : (u + 1) * WHD] = p_up[:, u * WHD + PERM64]

    # pad_v key-tiled [B, 128, 12, 64]: cols 64:128 (non-whisper dims) and
    # cols 0:64 (whisper dims, used as additive base on non-whisper cores)
    pv_pad = np.zeros((B, AT2, HS), f32)
    pv_pad[:, :AT, :] = pad_v
    pv_tiles = pv_pad.reshape(B, NKT, 128, HS).transpose(0, 2, 1, 3)
    padvT_hi = np.ascontiguousarray(pv_tiles[:, :, :, WHD:]).astype(NBF)
    padvT_lo = np.ascontiguousarray(pv_tiles[:, :, :, :WHD]).astype(NBF)
    padvT_lo_zero = np.zeros_like(padvT_lo)

    in_maps = []
    for c in range(NCORES):
        heads = range(HPC * c, HPC * c + HPC)
        wq_c = np.empty((C, HPC * HS), f32)
        wk_c = np.empty((C, HPC * HS), f32)
        wv_c = np.empty((C, HPC * HS), f32)
        akT_c = np.empty((HPC, HS, A_LEN), f32)
        av_c = np.empty((HPC, A_LEN, HS), f32)
        for hl, h in enumerate(heads):
            wq_c[:, hl * HS : (hl + 1) * HS] = c_attn[:, h * HS + PERM]
            wk_c[:, hl * HS : (hl + 1) * HS] = c_attn[:, C + h * HS + PERM]
            wv_c[:, hl * HS : (hl + 1) * HS] = (
                c_attn[:, 2 * C + h * HS : 2 * C + (h + 1) * HS]
            )
            akT_c[hl] = ak[:, h, PERM].T
            av_c[hl] = av[:, h, :]

        wk_core = c * HPC + HPC - 1 < NWH  # all 4 heads whisper-backed
        padkT_c = np.zeros((B, HS, AT2), f32)
        padkT_c[:, :, :AT] = padkT_perm
        if wk_core:
            aT_c = np.empty((AD, B * 300), f32)
            for b in range(B):
                aT_c[:, b * 300 : (b + 1) * 300] = aT_full[
                    :, b * AT + 300 * c : b * AT + 300 * c + 300
                ]
            pupk_c, pupv_c = pupk_all, p_up
            padkT_c[:, 0:32, :] = 0.0
            padkT_c[:, 64:96, :] = 0.0
            padv0_c = padvT_lo_zero
        else:
            aT_c = np.zeros((AD, B * 300), f32)
            pupk_c = np.zeros((DD, 20 * WHD), f32)
            pupv_c = np.zeros((DD, AD), f32)
            padv0_c = padvT_lo

        wq_t = np.ascontiguousarray(
            wq_c.astype(NBF).reshape(KO, 128, HPC, HS).transpose(2, 1, 0, 3)
        )
        wk_t = np.ascontiguousarray(
            wk_c.astype(NBF).reshape(KO, 128, HPC, HS).transpose(2, 1, 0, 3)
        )
        wv_t = np.ascontiguousarray(
            wv_c.astype(NBF).reshape(KO, 128, HPC * HS).transpose(1, 0, 2)
        )
        in_maps.append(
            dict(
                xT=xT,
                wq=wq_t, wk=wk_t, wv=wv_t,
                cosT=cosT, sinT=sinT, masks=masks,
                akT=akT_c.astype(NBF), avd=av_c.astype(NBF),
                aT=aT_c.astype(NBF),
                wkey=wkey_t, wval=wval_t,
                vbias=vb_t, rmsk=rmsk_t, rmsv=rmsv_t,
                pdown=np.ascontiguousarray(
                    np.stack([p_down * rms_key[:, None], p_down * rms_val[:, None]])
                ).astype(NBF),
                pupk=pupk_c.astype(NBF), pupv=pupv_c.astype(NBF),
                padkT=padkT_c.astype(NBF),
                padvT=padvT_hi, padv0=padv0_c,
                cproj=cproj_b,
            )
        )
    return in_maps


def get_program(inputs):
    gf = float(np.asarray(inputs["gating_factor"], np.float32))
    pg = float(np.asarray(inputs["proj_gating"], np.float32))
    key = (gf, pg)
    if key not in _PROG_CACHE:
        _PROG_CACHE[key] = build_program(gf, pg)
    return _PROG_CACHE[key]


def kernel(**inputs) -> np.ndarray:
    nc = get_program(inputs)
    in_maps = prepare_inputs(inputs)
    res = run_bass_kernel_spmd(nc, in_maps, core_ids=list(range(NCORES)))
    # core j rows 0:128 = batch-0 tokens [128j, 128j+128); rows 128:256 = batch 1
    full = np.empty((B, T, C), np.float32)
    for c in range(NCORES):
        r = res.results[c]["out"]
        full[0, 128 * c : 128 * (c + 1)] = r[0:128]
        full[1, 128 * c : 128 * (c + 1)] = r[128:256]
    return full



# revision 51
# speedup vs baseline: 1.0061x; 1.0061x over previous
"""Trainium2 Bass kernel for nn_CausalSelfAttention_90168543412719.

Sharding: head-parallel over the 32 attention heads (4 heads/core on 8
NeuronCores). Each core computes q/k/v projections for its heads from the
full x, runs causal + adapter-prefix + whisper cross attention for its
heads, then a per-batch AllToAll reshards y from head-sharded to
token-sharded and each core applies c_proj to its own 256 token rows.
Whisper K/V MLP is row-sharded across the 5 whisper cores.

All matmuls run in bf16 with fp32 PSUM accumulation. Host pre-slices /
pre-transposes / pre-casts every operand into the exact layout the PE
wants (x / qkv / whisper / cproj weights are host-pre-tiled so device DMAs
are contiguous slabs), so the device never transposes anything.

Phase order puts the qkv GEMM (pure PE work) first so the whisper-MLP
weight streams prefetch underneath it. Whisper keys are padded 1500->1536
so every kv loop runs 12 uniform 128-key tiles; the 36 tail keys are
killed with a per-partition bias of -30000 on the exp of the last tile.

Rope layout trick: the q/k head dims are permuted to [evens..., odds...]
(host permutes the corresponding weight columns), so rope becomes four
contiguous 64-partition block ops. Scores contract over the permuted dim
on both sides, so the permutation cancels; v / y stay in natural order.

Attention works in transposed score space: s_T[keys, q] = k_T.T @ q_T, so
probabilities come out in the exact [keys, q] layout the AV matmul wants
as rhs (no P transposes). Softmax denominators are column sums computed
on the PE with a ones vector; no max-shift is needed at these scales.

Performance structure (evidence from NTFF/perfetto profiles):
- Softmax denominators for a unit land on PSUM partitions {0,32,64,96}
  of ONE bank, so a single batched DVE reciprocal serves all branches;
  probability tiles are pairwise-summed on the DVE so only one
  denominator matmul runs per two key tiles.
- Filler PE work (whisper MLP, pv up-projection, pk assembly) is emitted
  BETWEEN a unit's denominator completion and its normalize matmuls, so
  the PE never waits on the DVE reciprocal chain.
- SBUF-only elementwise work (pk/pv adds, adapter normalize, whisper h
  scaling) runs on the otherwise-idle Pool engine; PSUM evacuations pick
  whichever of ACT/DVE is cold in that phase.
- The AllToAll payload is staged [dest, dim, head, tok] so the receive
  gather is 1KB-contiguous; c_proj weights stream through a 4-deep
  half-tile ring, each column block fetched ONCE and consumed by both
  token halves back-to-back (tail is PE-bound, zero steady-state gaps).
- kT / v / rope tables / A1-only whisper constants live in right-side
  stack pools that die after the causal phase, freeing SBUF for the
  c_proj ring; rms weights are folded into proj_down on the host.
"""

import os
import sys
from contextlib import ExitStack

import numpy as np
import ml_dtypes

for _p in ("/root/.axon_site/_ro/trn_rl_repo", "/opt/trn_rl_repo"):
    if os.path.isdir(_p) and _p not in sys.path:
        sys.path.append(_p)

import concourse.bass as bass
import concourse.mybir as mybir
import concourse.tile as tile
from concourse.bass_utils import run_bass_kernel_spmd

BF16 = mybir.dt.bfloat16
F32 = mybir.dt.float32
NBF = ml_dtypes.bfloat16
AF = mybir.ActivationFunctionType
ALU = mybir.AluOpType

B, T, C = 2, 1024, 4096
NH, HS = 32, 128
NCORES, HPC = 8, 4  # heads per core
A_LEN = 10
AT, AD, DD = 1500, 1280, 80  # audio_t, audio_d, down dim
AT2 = 1536  # whisper keys padded to 12*128
NWH, WHD = 20, 64  # whisper heads / head dim
EPS = 1e-5
BT = B * T  # 2048 global tokens, b-major
TT = 512  # token tile (matmul free dim)
NTT = BT // TT  # 4
TPC = BT // NCORES  # 256 tokens per core for c_proj
SCALE = 1.0 / float(np.sqrt(HS))
NEG = -30000.0  # additive mask value pre-scale; exp(NEG*SCALE) == 0 in f32
NKT = AT2 // 128  # 12 whisper key tiles per batch
KO = C // 128  # 32 contraction tiles over C
NOT = AD // 128  # 10 whisper tiles over AD

PERM = np.concatenate([np.arange(0, HS, 2), np.arange(1, HS, 2)])  # 128
PERM64 = np.concatenate([np.arange(0, WHD, 2), np.arange(1, WHD, 2)])  # 64

_PROG_CACHE = {}
_MAX_WAITS = 1


def _split_multi_waits(nc):
    """walrus here rejects >1 semaphore wait per instruction; hoist extras
    onto preceding NoOps on the same engine."""
    for f in nc.m.functions:
        for blk in f.blocks:
            insts = list(blk.instructions)
            new = []
            changed = False
            for inst in insts:
                si = inst.sync_info
                if si is not None and si.on_wait and len(si.on_wait) > _MAX_WAITS:
                    waits = list(si.on_wait)
                    keep = waits[-_MAX_WAITS:]
                    extra = waits[:-_MAX_WAITS]
                    for i in range(0, len(extra), _MAX_WAITS):
                        new.append(
                            mybir.InstNoOp(
                                name=f"{inst.name}.wsplit{i}",
                                engine=inst.engine,
                                debug=inst.debug,
                                sync_info=mybir.SyncInfo(
                                    on_wait=extra[i : i + _MAX_WAITS], on_update=[]
                                ),
                                bass_nofuse=True,
                            )
                        )
                    inst.sync_info = mybir.SyncInfo(
                        on_wait=keep, on_update=list(si.on_update)
                    )
                    changed = True
                new.append(inst)
            if changed:
                try:
                    blk.instructions[:] = new
                except TypeError:
                    blk.instructions = new


def build_program(gating_factor: float, proj_gating: float) -> bass.Bass:
    nc = bass.Bass()

    # ---------------- I/O (per-core data arrives via in_maps)
    xT = nc.dram_tensor("xT", [NTT, 128, KO, TT], BF16, kind="ExternalInput")
    wq = nc.dram_tensor("wq", [HPC, 128, KO, HS], BF16, kind="ExternalInput")
    wk = nc.dram_tensor("wk", [HPC, 128, KO, HS], BF16, kind="ExternalInput")
    wv = nc.dram_tensor("wv", [128, KO, HPC * HS], BF16, kind="ExternalInput")
    cosT = nc.dram_tensor("cosT", [HS // 2, T], F32, kind="ExternalInput")
    sinT = nc.dram_tensor("sinT", [HS // 2, T], F32, kind="ExternalInput")
    masks = nc.dram_tensor("masks", [128, 128], F32, kind="ExternalInput")
    akT = nc.dram_tensor("akT", [HPC, HS, A_LEN], BF16, kind="ExternalInput")
    avd = nc.dram_tensor("avd", [HPC, A_LEN, HS], BF16, kind="ExternalInput")
    aTd = nc.dram_tensor("aT", [AD, B * 300], BF16, kind="ExternalInput")
    wkey = nc.dram_tensor("wkey", [NOT, 128, NOT, 128], BF16, kind="ExternalInput")
    wval = nc.dram_tensor("wval", [NOT, 128, NOT, 128], BF16, kind="ExternalInput")
    vbias = nc.dram_tensor("vbias", [128, NOT], F32, kind="ExternalInput")
    rmsk = nc.dram_tensor("rmsk", [128, NOT], F32, kind="ExternalInput")
    rmsv = nc.dram_tensor("rmsv", [128, NOT], F32, kind="ExternalInput")
    pdown = nc.dram_tensor("pdown", [2, AD, DD], BF16, kind="ExternalInput")
    pupk = nc.dram_tensor("pupk", [DD, 20 * WHD], BF16, kind="ExternalInput")
    pupv = nc.dram_tensor("pupv", [DD, AD], BF16, kind="ExternalInput")
    padkT = nc.dram_tensor("padkT", [B, HS, AT2], BF16, kind="ExternalInput")
    padvT = nc.dram_tensor("padvT", [B, 128, NKT, WHD], BF16, kind="ExternalInput")
    padv0 = nc.dram_tensor("padv0", [B, 128, NKT, WHD], BF16, kind="ExternalInput")
    cproj = nc.dram_tensor("cproj", [C // TT, KO, 128, TT], BF16, kind="ExternalInput")
    out = nc.dram_tensor("out", [TPC, C], F32, kind="ExternalOutput")

    gf = float(gating_factor)
    pg = float(proj_gating)

    with tile.TileContext(nc) as tc, ExitStack() as ctx:
        dram = ctx.enter_context(tc.tile_pool(name="dram", bufs=1, space="DRAM"))
        const = ctx.enter_context(tc.tile_pool(name="const", bufs=1))
        persist = ctx.enter_context(tc.tile_pool(name="persist", bufs=1))

        # Collective bounce (split per batch) + whisper pv staging in DRAM
        a2a0_in = dram.tile([NCORES, HS, HPC, 128], BF16)
        a2a0_out = dram.tile([NCORES, HS, HPC, 128], BF16)
        a2a1_in = dram.tile([NCORES, HS, HPC, 128], BF16)
        a2a1_out = dram.tile([NCORES, HS, HPC, 128], BF16)
        a2a_ins = [a2a0_in, a2a1_in]
        a2a_outs = [a2a0_out, a2a1_out]
        pv_d = dram.tile([B, HPC, AT2 * WHD], BF16)  # per-(b,head) flat pv rows

        ones_bf = const.tile([128, 1], BF16)
        nc.gpsimd.memset(ones_bf[:], 1.0)
        ones_row = const.tile([1, 128], BF16)
        nc.gpsimd.memset(ones_row[:], 1.0)
        ones128 = const.tile([128, 128], BF16)
        nc.gpsimd.memset(ones128[:], 1.0)
        ones128f = const.tile([128, 128], F32)
        nc.gpsimd.memset(ones128f[:], 1.0)
        eps_sb = const.tile([1, 1], F32)
        nc.gpsimd.memset(eps_sb[:], EPS)
        tailb = const.tile([128, 1], F32)  # kill keys 1500:1536 in last tile
        nc.gpsimd.memset(tailb[:], NEG)
        nc.gpsimd.memset(tailb[0 : AT - 11 * 128, :], 0.0)
        zrow = const.tile([AT2 - AT, WHD], BF16)
        nc.gpsimd.memset(zrow[:], 0.0)

        # SBUF state persisting through attention (freed before phase P)
        mid = ctx.enter_context(ExitStack())
        midp = mid.enter_context(tc.tile_pool(name="midp", bufs=1))
        qT_sb = midp.tile([128, HPC, NTT, TT], BF16)  # roped q, permuted dims
        # right-side pools die after A1, freeing SBUF for the c_proj
        # weight ring before it opens (stack allocator is per-side LIFO)
        rts = ExitStack()
        kTp = rts.enter_context(tc.tile_pool(name="kTp", bufs=1, side="right"))
        kT_sb = kTp.tile([128, HPC, NTT, TT], BF16)  # roped k, permuted dims
        v_sb = kTp.tile([128, NTT, 4, HPC * HS], BF16)  # [tok128, tt, st, cols]
        cos_sb = kTp.tile([64, T], F32)
        sin_sb = kTp.tile([64, T], F32)
        mask_sb = kTp.tile([128, 128], F32)
        akT_sb = const.tile([128, HPC, A_LEN], BF16)
        av_sb = const.tile([A_LEN, HPC, HS], BF16)
        dk_loc = persist.tile([DD, B * 300], BF16)  # whisper down-proj, own rows
        dv_loc = persist.tile([DD, B * 300], BF16)

        # W1 constants (outside Q's pools so the DMAs overlap Q). The
        # A1-only set lives on the right side and dies with the fillers;
        # pupv_sb survives into A2-b0 (w2_unit(1, *)) so it stays in whc.
        whc = mid.enter_context(tc.tile_pool(name="whc", bufs=1))
        whcA = rts.enter_context(tc.tile_pool(name="whcA", bufs=1, side="right"))
        aT_sb = whcA.tile([128, NOT, B * 300], BF16)
        pdown_sb = whcA.tile([128, 2, NOT, DD], BF16)
        vb_sb = whcA.tile([128, NOT], F32)
        pupv_sb = whc.tile([DD, AD], BF16)

        def deferred_const_dmas():
            # issued after the first Q tiles so the critical first matmul
            # chain is not starved by prefetch traffic; rope needs cos/sin
            # early, the rest is small
            nc.sync.dma_start(cos_sb[:], cosT[:])
            nc.sync.dma_start(sin_sb[:], sinT[:])
            nc.sync.dma_start(mask_sb[:], masks[:])
            nc.sync.dma_start(vb_sb[:], vbias[:])

        def deferred_const_dmas2():
            # the bulky whisper/adapter constants wait until the second x
            # tile is in flight so they don't delay Q's weight prefetches
            nc.sync.dma_start(akT_sb[:], akT[:].rearrange("h p a -> p h a"))
            nc.sync.dma_start(av_sb[:], avd[:].rearrange("h a d -> a h d"))
            nc.sync.dma_start(aT_sb[:], aTd[:].rearrange("(ko p) r -> p ko r", p=128))
            nc.sync.dma_start(
                pdown_sb[:], pdown[:].rearrange("v (ko p) n -> p v ko n", p=128)
            )
            nc.sync.dma_start(pupv_sb[:], pupv[:])

        # =============== Phase Q: qkv projection + rope. Token tiles
        # are processed in PAIRS so each q/k weight LDWEIGHTS serves two
        # matmuls (the exposed weight-load gap halves). Per pair the
        # order is v(first tile) -> q/k(both) -> v(second tile), so the
        # second tile's x stream and the next pair's loads always hide
        # under ~35us of compute.
        with (
            tc.tile_pool(name="qx", bufs=2) as qx,
            tc.tile_pool(name="qw", bufs=2) as qw,
            tc.tile_pool(name="qwv", bufs=1) as qwv,
            tc.tile_pool(name="qpk", bufs=4, space="PSUM") as qpk,
            tc.tile_pool(name="qpv", bufs=4, space="PSUM") as qpv,
            tc.tile_pool(name="qt", bufs=2) as qtp,
        ):
            wv_w = qwv.tile([128, KO, HPC * HS], BF16)

            def rope(ps, dst, hl, tt):
                co = (tt % 2) * TT  # rope position offset within batch
                ev, od = ps[0:64, :], ps[64:128, :]
                cs = cos_sb[:, co : co + TT]
                sn = sin_sb[:, co : co + TT]
                t1 = qtp.tile([64, TT], F32, tag="r1")
                t2 = qtp.tile([64, TT], F32, tag="r2")
                nc.vector.tensor_tensor(t1[:], ev, cs, ALU.mult)
                nc.vector.tensor_tensor(t2[:], od, sn, ALU.mult)
                nc.vector.tensor_sub(dst[0:64, hl, tt, :], t1[:], t2[:])
                nc.vector.tensor_tensor(t1[:], od, cs, ALU.mult)
                nc.vector.tensor_tensor(t2[:], ev, sn, ALU.mult)
                nc.vector.tensor_add(dst[64:128, hl, tt, :], t1[:], t2[:])

            def v_proj(x_t, tt):
                # ko-outer with 4 st banks live so consumption tracks the
                # chunked x / wv streams ko-progressively
                pss = []
                for _i in range(4):
                    v_ps = qpv.tile([128, HPC * HS], F32, tag="v_ps")
                    pss.append(v_ps)
                for ko in range(KO):
                    for st in range(4):
                        nc.tensor.matmul(
                            pss[st][:],
                            x_t[:, ko, st * 128 : (st + 1) * 128],
                            wv_w[:, ko, :],
                            start=(ko == 0), stop=(ko == KO - 1),
                        )
                for st in range(4):
                    nc.scalar.copy(v_sb[:, tt, st, :], pss[st][:])

            for pair in range(NTT // 2):
                x_ts = []
                for j in range(2):
                    x_t = qx.tile([128, KO, TT], BF16, tag="x_t")
                    x_ts.append(x_t)
                # chunked, interleaved loads: first-tile and wv chunks
                # lead so v(first) starts immediately; second-tile chunks
                # stream underneath it
                for k0 in range(0, KO, 8):
                    nc.sync.dma_start(
                        x_ts[0][:, k0 : k0 + 8, :], xT[2 * pair, :, k0 : k0 + 8, :]
                    )
                    if pair == 0:
                        nc.sync.dma_start(
                            wv_w[:, k0 : k0 + 8, :], wv[:, k0 : k0 + 8, :]
                        )
                for k0 in range(0, KO, 8):
                    nc.sync.dma_start(
                        x_ts[1][:, k0 : k0 + 8, :], xT[2 * pair + 1, :, k0 : k0 + 8, :]
                    )
                if pair == 0:
                    deferred_const_dmas()
                v_proj(x_ts[0], 2 * pair)
                for ph in range(2):  # 0: q, 1: k
                    wsrc = wq if ph == 0 else wk
                    dst = qT_sb if ph == 0 else kT_sb
                    for hl in range(HPC):
                        w_t = qw.tile([128, KO, HS], BF16, tag="w_t")
                        nc.sync.dma_start(w_t[:], wsrc[hl])
                        if pair == 0 and ph == 1 and hl == 0:
                            deferred_const_dmas2()
                        # one LDWEIGHTS per ko serves both token tiles
                        pss = []
                        for _i in range(2):
                            qk_ps = qpk.tile([128, TT], F32, tag="qk_ps")
                            pss.append(qk_ps)
                        for ko in range(KO):
                            for j in range(2):
                                nc.tensor.matmul(
                                    pss[j][:], w_t[:, ko, :], x_ts[j][:, ko, :],
                                    start=(ko == 0), stop=(ko == KO - 1),
                                )
                        for j in range(2):
                            rope(pss[j], dst, hl, 2 * pair + j)
                v_proj(x_ts[1], 2 * pair + 1)

        # causal+adapter partial y, held until the whisper branch adds in.
        # Allocated after Q's pools close so it reuses their SBUF space.
        ostp = mid.enter_context(tc.tile_pool(name="ostp", bufs=1))
        o_store = ostp.tile([128, B * HPC * 2, TT], BF16)

        # =============== Phases A1+A2+P: attention + c_proj. A1 does
        # causal+adapter into o_store, with whisper-MLP (W1), pv
        # up-projection (W2) and the b=0 pk assembly interleaved as filler
        # work. A2 does whisper cross-attention, with b=1 prep interleaved
        # into b=0's slots and c_proj batch-0 chunks woven into b=1's
        # slots; each batch's AllToAll launches as soon as it is staged.
        # Softmax denominators for a unit land on PSUM partitions
        # {0,32,64,96} of one bank so ONE reciprocal + ONE cast serve all
        # branches, and filler PE work is emitted between the denominator
        # completion and the normalize matmuls so the PE never waits on
        # the DVE reciprocal chain.
        with (
            tc.tile_pool(name="w2", bufs=2) as w2,
            tc.tile_pool(name="ap", bufs=2) as ap,
            tc.tile_pool(name="apk", bufs=2) as apk,
            tc.tile_pool(name="apv", bufs=2) as apv,
            tc.tile_pool(name="pt2", bufs=6) as pt2,
            tc.tile_pool(name="ascp", bufs=2, space="PSUM") as ascp,
            tc.tile_pool(name="ayp", bufs=2, space="PSUM") as ayp,
            tc.tile_pool(name="adp", bufs=2, space="PSUM") as adp,
        ):
            # W1-only pools live in a nested scope freed after the A1
            # fillers drain, returning 2 PSUM banks + ~25KB/part of SBUF.
            w1s = ExitStack()
            wh = w1s.enter_context(tc.tile_pool(name="wh", bufs=1))
            whs = w1s.enter_context(tc.tile_pool(name="whs", bufs=2))
            whp_h = w1s.enter_context(tc.tile_pool(name="whp_h", bufs=1, space="PSUM"))
            whp_s = w1s.enter_context(tc.tile_pool(name="whp_s", bufs=1, space="PSUM"))
            whp_m = whp_h
            pupk_sb = apk.tile([DD, 20, WHD], BF16, tag="pupk")
            nc.sync.dma_start(pupk_sb[:], pupk[:].rearrange("d (u i) -> d u i", i=WHD))

            w1_state = {}
            pk4_t = {}

            def prefetch_w(kv, ot):
                w_t = whs.tile([128, NOT, 128], BF16, tag="wh_w")
                w_dram = wkey if kv == 0 else wval
                nc.sync.dma_start(w_t[:], w_dram[ot])
                w1_state["w_next"] = w_t

            def h_unit(kv, ot):
                w_t = w1_state["w_next"]
                if ot + 1 < NOT:
                    prefetch_w(kv, ot + 1)
                elif kv == 0:
                    prefetch_w(1, 0)
                if ot == 0:
                    h_cur = wh.tile([128, NOT, B * 300], BF16, tag="h_sb")
                    ssq_cur = whp_s.tile([33, 300], F32, tag="ssq")
                    w1_state["h"] = h_cur
                    w1_state["ssq"] = ssq_cur
                h_sb = w1_state["h"]
                ssq = w1_state["ssq"]
                for b2 in range(2):
                    c0 = 300 * b2
                    hp_t = whp_h.tile([128, TT], F32, tag="m")
                    hp = hp_t[:, 0:300]
                    for kt in range(NOT):
                        nc.tensor.matmul(
                            hp,
                            w_t[:, kt, :],
                            aT_sb[:, kt, c0 : c0 + 300],
                            start=(kt == 0), stop=(kt == NOT - 1),
                        )
                    if kv == 1:
                        nc.scalar.activation(
                            h_sb[:, ot, c0 : c0 + 300], hp, AF.Identity,
                            bias=vb_sb[:, ot : ot + 1],
                        )
                    else:
                        nc.scalar.copy(h_sb[:, ot, c0 : c0 + 300], hp)
                    hsq = wh.tile([128, 300], BF16, tag="hsq")
                    nc.scalar.activation(hsq[:], h_sb[:, ot, c0 : c0 + 300], AF.Square)
                    sr = 32 * b2
                    nc.tensor.matmul(
                        ssq[sr : sr + 1, :], ones_bf[:], hsq[:],
                        start=(ot == 0), stop=(ot == NOT - 1),
                        skip_group_check=True,
                    )

            def w1_tail(kv):
                d_dst = dk_loc if kv == 0 else dv_loc
                h_sb = w1_state["h"]
                ssq = w1_state["ssq"]
                for b2 in range(2):
                    c0 = 300 * b2
                    sq_sb = wh.tile([1, 300], F32, tag="sq_sb")
                    sr = 32 * b2
                    nc.scalar.activation(
                        sq_sb[:], ssq[sr : sr + 1, :], AF.Sqrt,
                        bias=eps_sb[:], scale=1.0 / AD,
                    )
                    rr_sb = wh.tile([1, 300], F32, tag="rr_sb")
                    rr_bf = wh.tile([1, 300], BF16, tag="rr_bf")
                    nc.vector.reciprocal(rr_sb[:], sq_sb[:])
                    nc.scalar.copy(rr_bf[:], rr_sb[:])
                    m1 = whp_m.tile([128, TT], F32, tag="m")
                    nc.tensor.matmul(
                        m1[:, 0:300], ones_row[:], rr_bf[:], start=True, stop=True
                    )
                    rrb = wh.tile([128, 300], F32, tag="rrb")
                    nc.scalar.copy(rrb[:], m1[:, 0:300])
                    hn_sb = wh.tile([128, NOT, 300], BF16, tag="hn_sb")
                    for ot in range(NOT):
                        # rms weights are folded into pdown on the host, so
                        # this is a plain multiply the idle Pool engine can do
                        nc.gpsimd.tensor_tensor(
                            hn_sb[:, ot, :], h_sb[:, ot, c0 : c0 + 300],
                            rrb[:], ALU.mult,
                        )
                    m2 = whp_m.tile([128, TT], F32, tag="m")
                    for kt in range(NOT):
                        nc.tensor.matmul(
                            m2[0:DD, 0:300], pdown_sb[:, kv, kt, :], hn_sb[:, kt, :],
                            start=(kt == 0), stop=(kt == NOT - 1),
                        )
                    nc.scalar.activation(d_dst[:, c0 : c0 + 300], m2[0:DD, 0:300], AF.Silu)

            def w2_unit(b, hl):
                # pv rows: head g keys [1500, 64] are wv rows [75g, 75g+75)
                # flat row-major; rows 1500:1536 (tail) zeroed.
                wvrow = w2.tile([128, AD], BF16, tag="wvrow")
                for ns in range(3):
                    n0 = ns * 512
                    nsz = min(512, AD - n0)
                    ps = ascp.tile([128, TT], F32, tag="sc")
                    nc.tensor.matmul(
                        ps[0:75, :nsz],
                        dv_loc[:, b * 300 + 75 * hl : b * 300 + 75 * (hl + 1)],
                        pupv_sb[:, n0 : n0 + nsz],
                        start=True, stop=True,
                    )
                    nc.scalar.copy(wvrow[0:75, n0 : n0 + nsz], ps[0:75, :nsz])
                nc.sync.dma_start(
                    pv_d[b, hl, : AT * WHD].rearrange("(r d) -> r d", r=75),
                    wvrow[0:75, :],
                )
                nc.sync.dma_start(
                    pv_d[b, hl, AT * WHD :].rearrange("(r d) -> r d", r=AT2 - AT),
                    zrow[:],
                )

            def pk4_start(b):
                # pk for all 4 heads: [128 dims, hl, 1536 keys].
                # pk[i, hl, 20*jr+u] = pad[i, key] (whisper rows host-zeroed)
                #                    + pupk[:, u, i] . dk[:, 75*hl+jr]
                pk4 = apk.tile([128, HPC, AT2], BF16, tag="pk4")
                pk4_t[b] = pk4
                for hl in range(HPC):
                    nc.sync.dma_start(pk4[:, hl, :], padkT[b, :, :])

            def pk4_chunk(b, u0):
                # PSUM is evacuated by the scalar engine and the adds run
                # on the idle gpsimd engine so they neither load the DVE
                # (whose queue backs up behind the unit tails) nor
                # head-of-line-block the PE stream
                pk4 = pk4_t[b]
                pk4v = pk4[:, :, :AT].rearrange("p h (j u) -> p h j u", u=20)
                dk4 = dk_loc[:, b * 300 : (b + 1) * 300]
                for u in range(u0, u0 + 5):
                    # b=1 chunks run inside A2-b0 where the shared score
                    # tag's consumers lag on the busy ACT/DVE queues; a
                    # dedicated bank decouples them from the PE stream
                    pool = pkq if b == 1 else ascp
                    tag = "pk" if b == 1 else "sc"
                    pkp = pool.tile([128, TT], F32, tag=tag)
                    nc.tensor.matmul(
                        pkp[0:32, 0:300], pupk_sb[:, u, 0:32], dk4,
                        start=True, stop=True,
                    )
                    nc.tensor.matmul(
                        pkp[64:96, 0:300], pupk_sb[:, u, 32:64], dk4,
                        start=True, stop=True,
                    )
                    pk_sb = w2.tile([128, 300], F32, tag="pksb")
                    copy = nc.scalar.copy if b == 0 else nc.vector.tensor_copy
                    for half in range(2):
                        p0 = 64 * half
                        copy(pk_sb[p0 : p0 + 32, :], pkp[p0 : p0 + 32, 0:300])
                    for half in range(2):
                        p0 = 64 * half
                        nc.gpsimd.tensor_add(
                            pk4v[p0 : p0 + 32, :, :, u],
                            pk_sb[p0 : p0 + 32, :].rearrange(
                                "p (h j) -> p h j", h=HPC
                            ),
                            pk4v[p0 : p0 + 32, :, :, u],
                        )

            prefetch_w(0, 0)
            fillers = []
            for ot in range(NOT):
                fillers.append(lambda ot=ot: h_unit(0, ot))
            fillers.append(lambda: w1_tail(0))
            # pk assembly right after dk is ready so the gpsimd add chain
            # drains long before A2's first scores read pk4
            fillers.append(lambda: pk4_start(0))
            for u0 in range(0, 20, 5):
                fillers.append(lambda u0=u0: pk4_chunk(0, u0))
            for ot in range(NOT):
                fillers.append(lambda ot=ot: h_unit(1, ot))
            fillers.append(lambda: w1_tail(1))
            for hl in range(HPC):
                fillers.append(lambda hl=hl: w2_unit(0, hl))

            n_units = B * HPC
            fidx = 0

            # ---------------- A1: causal + adapter. Denominators land on
            # PSUM rows {0: causal qt0, 32: causal qt1, 64: adapter qt0,
            # 96: adapter qt1} of one bank; one batched reciprocal + cast
            # serves all four, with filler PE work emitted in between so
            # the normalize matmuls never wait on the DVE.
            for b in range(B):
                for hl in range(HPC):
                    dt = adp.tile([97, TT], F32, tag="den")
                    ys = {}
                    ya_sbs = {}
                    for qt in range(2):
                        qcol = qT_sb[:, hl, 2 * b + qt, :]  # [128, 512]
                        nkt = 4 * (qt + 1)
                        y_ps = ayp.tile([128, TT], F32, tag="y")
                        dr = 32 * qt
                        pts = {}
                        for kt in range(nkt):
                            sp = ascp.tile([128, TT], F32, tag="sc")
                            nc.tensor.matmul(
                                sp[:],
                                kT_sb[:, hl, 2 * b + kt // 4,
                                      (kt % 4) * 128 : (kt % 4) * 128 + 128],
                                qcol, start=True, stop=True,
                            )
                            pt = pt2.tile([128, TT], BF16, tag="pt2")
                            roff = kt * 128 - qt * TT
                            if roff >= 0:  # diagonal block
                                # columns < roff are fully masked: zero them
                                # in pt (off the critical path); add the
                                # shared [128,128] triangular mask on the
                                # boundary column block; exp the rest.
                                if roff > 0:
                                    nc.gpsimd.memset(pt[:, 0:roff], 0.0)
                                nc.vector.tensor_add(
                                    sp[:, roff : roff + 128],
                                    sp[:, roff : roff + 128],
                                    mask_sb[:],
                                )
                                nc.scalar.activation(
                                    pt[:, roff:TT], sp[:, roff:TT], AF.Exp, scale=SCALE
                                )
                            else:
                                nc.scalar.activation(pt[:], sp[:], AF.Exp, scale=SCALE)
                            pts[kt] = pt
                            if kt >= 1:
                                ptp = pts[kt - 1]
                                nc.tensor.matmul(
                                    y_ps[:],
                                    v_sb[:, 2 * b + (kt - 1) // 4, (kt - 1) % 4,
                                         hl * HS : (hl + 1) * HS],
                                    ptp[:],
                                    start=(kt - 1 == 0), stop=False,
                                )
                                if kt % 2 == 0:
                                    pa_, pb_ = pts.pop(kt - 2), pts.pop(kt - 1)
                                    psum2 = pt2.tile([128, TT], BF16, tag="ptsum")
                                    nc.vector.tensor_add(psum2[:], pa_[:], pb_[:])
                                    nc.tensor.matmul(
                                        dt[dr : dr + 1, :], ones_bf[:], psum2[:],
                                        start=(kt == 2), stop=False,
                                        skip_group_check=True,
                                    )
                        ptp = pts[nkt - 1]
                        nc.tensor.matmul(
                            y_ps[:],
                            v_sb[:, 2 * b + (nkt - 1) // 4, (nkt - 1) % 4,
                                 hl * HS : (hl + 1) * HS],
                            ptp[:],
                            start=False, stop=True,
                        )
                        pa_, pb_ = pts.pop(nkt - 2), pts.pop(nkt - 1)
                        psum2 = pt2.tile([128, TT], BF16, tag="ptsum")
                        nc.vector.tensor_add(psum2[:], pa_[:], pb_[:])
                        nc.tensor.matmul(
                            dt[dr : dr + 1, :], ones_bf[:], psum2[:],
                            start=False, stop=True, skip_group_check=True,
                        )
                        ys[qt] = y_ps
                        # ---- adapter prefix attention: PE work inline;
                        # ya is evacuated to SBUF so its PSUM bank frees
                        # and normalization can defer to the unit tail.
                        sa = ascp.tile([128, TT], F32, tag="sc")
                        nc.tensor.matmul(
                            sa[0:A_LEN, :], akT_sb[:, hl, :], qcol, start=True, stop=True
                        )
                        pa = ap.tile([A_LEN, TT], BF16, tag="pa")
                        nc.scalar.activation(pa[:], sa[0:A_LEN, :], AF.Exp, scale=SCALE)
                        nc.tensor.matmul(
                            dt[64 + dr : 64 + dr + 1, :], ones_bf[0:A_LEN, :], pa[:],
                            start=True, stop=True, skip_group_check=True,
                            tile_position=(0, 64 + dr),
                        )
                        ya = ascp.tile([128, TT], F32, tag="sc")
                        nc.tensor.matmul(ya[:], av_sb[:, hl, :], pa[:], start=True, stop=True)
                        ya_sb = ap.tile([128, TT], F32, tag="yasb")
                        nc.scalar.copy(ya_sb[:], ya[:])
                        ya_sbs[qt] = ya_sb

                    # one reciprocal for all four denominators; the bf16
                    # cast runs on the scalar engine (fp32 rep matmuls run
                    # LOW_HIGH at 2x cost, so bf16 reps win)
                    rc = ap.tile([97, TT], F32, tag="rc")
                    rc_bf = ap.tile([97, TT], BF16, tag="rcbf")
                    nc.vector.reciprocal(rc[:], dt[:])
                    nc.scalar.copy(rc_bf[:], rc[:])

                    # filler PE work hides the recip+cast latency
                    unit = b * HPC + hl
                    want = ((unit + 1) * len(fillers)) // n_units
                    mid_stop = min(max(fidx + 2, want - 1), len(fillers))
                    while fidx < mid_stop:
                        fillers[fidx]()
                        fidx += 1

                    for qt in range(2):
                        oi = (b * HPC + hl) * 2 + qt
                        dr = 32 * qt
                        rep2 = ascp.tile([128, TT], F32, tag="sc")
                        nc.tensor.matmul(
                            rep2[:], ones128[64 + dr : 64 + dr + 1, :],
                            rc_bf[64 + dr : 64 + dr + 1, :],
                            start=True, stop=True, tile_position=(64 + dr, 0),
                        )
                        rep_sb2 = ap.tile([128, TT], F32, tag="repsb")
                        nc.scalar.copy(rep_sb2[:], rep2[:])
                        tmp = ap.tile([128, TT], F32, tag="tmp")
                        nc.gpsimd.tensor_tensor(tmp[:], ya_sbs[qt][:], rep_sb2[:], ALU.mult)
                        rep = ascp.tile([128, TT], F32, tag="sc")
                        nc.tensor.matmul(
                            rep[:], ones128[dr : dr + 1, :], rc_bf[dr : dr + 1, :],
                            start=True, stop=True,
                        )
                        rep_sb = ap.tile([128, TT], F32, tag="repsb")
                        nc.scalar.copy(rep_sb[:], rep[:])
                        o_sb = ap.tile([128, TT], F32, tag="o_sb")
                        nc.vector.tensor_tensor(o_sb[:], ys[qt][:], rep_sb[:], ALU.mult)
                        nc.vector.scalar_tensor_tensor(
                            o_store[:, oi, :], tmp[:], gf, o_sb[:], ALU.mult, ALU.add
                        )
                    while fidx < want:
                        fillers[fidx]()
                        fidx += 1
            while fidx < len(fillers):
                fillers[fidx]()
                fidx += 1
            w1s.close()
            rts.close()  # kT + A1-only whisper constants die here
            pps = ExitStack()
            pwp = pps.enter_context(tc.tile_pool(name="pw", bufs=4))
            py = pps.enter_context(tc.tile_pool(name="py", bufs=1))
            po = pps.enter_context(tc.tile_pool(name="po", bufs=2))
            pp = pps.enter_context(tc.tile_pool(name="pp", bufs=1, space="PSUM"))
            pkq = pps.enter_context(tc.tile_pool(name="pkq", bufs=1, space="PSUM"))

            # ---------------- P-phase machinery (c_proj). Batch-0 chunks
            # weave into A2-b1 as fillers; the weight ring re-streams
            # cproj for the batch-1 chunks at the end.
            NP = C // TT
            yT = {}

            def yT_load(half):
                t = py.tile([128, KO, 128], BF16, tag=f"yT{half}")
                nc.sync.dma_start(
                    t[:].rearrange("p (i h) t -> p i h t", h=HPC),
                    a2a_outs[half][:].rearrange("i d h t -> d i h t"),
                )
                yT[half] = t

            wring = {}
            KH = KO // 2

            def w_fetch(n, gen=0):
                # two half-tiles per column block so the ring stays at
                # 2x16KB/part and halves pipeline under the chunk matmuls
                for h in range(2):
                    w_h = pwp.tile([128, KH, TT], BF16, tag="w_n")
                    nc.sync.dma_start(
                        w_h[:],
                        cproj[n, h * KH : (h + 1) * KH, :, :].rearrange(
                            "ko p t -> p ko t"
                        ),
                    )
                    wring[(gen, n, h)] = w_h

            def p_chunk(n, half, gen=0, pop=True):
                ps = pp.tile([128, TT], F32, tag="o_ps")
                for h in range(2):
                    key = (gen, n, h)
                    w_h = wring.pop(key) if pop else wring[key]
                    for k in range(KH):
                        ko = h * KH + k
                        nc.tensor.matmul(
                            ps[:],
                            yT[half][:, ko, :],
                            w_h[:, k, :],
                            start=(ko == 0), stop=(ko == KO - 1),
                        )
                o_t = po.tile([128, TT], F32, tag="o_t")
                nc.scalar.copy(o_t[:], ps[:])
                nc.sync.dma_start(
                    out[half * 128 : (half + 1) * 128, n * TT : (n + 1) * TT], o_t[:]
                )

            # ---------------- A2: whisper cross attention
            a2f = []
            a2f.append(lambda: pk4_start(1))
            for u0 in range(0, 20, 5):
                a2f.append(lambda u0=u0: pk4_chunk(1, u0))
            for hl in range(HPC):
                a2f.append(lambda hl=hl: w2_unit(1, hl))
            a2n = 0

            for b in range(B):
                pk4 = pk4_t[b]
                for hl in range(HPC):
                    # pv [keys, kt, 128d]: cols 0:64 whisper rows (+pad for
                    # non-whisper cores via padv0), cols 64:128 pad.
                    pv4 = apv.tile([128, NKT, HS], BF16, tag="pv")
                    nc.sync.dma_start(
                        pv4[:, :, 0:WHD],
                        pv_d[b, hl, :].rearrange("(kt p d) -> p kt d", p=128, d=WHD),
                    )
                    nc.sync.dma_start(pv4[:, :, WHD:HS], padvT[b, :, :, :])
                    p0t = apv.tile([128, NKT, WHD], BF16, tag="p0t")
                    nc.sync.dma_start(p0t[:], padv0[b, :, :, :])
                    nc.gpsimd.tensor_add(pv4[:, :, 0:WHD], p0t[:], pv4[:, :, 0:WHD])

                    dt = adp.tile([97, TT], F32, tag="den")
                    # both query tiles' kt-loops interleaved: two independent
                    # score->exp->den/AV streams keep the PE fed while the
                    # Activation engine works through the exps
                    yw0 = ayp.tile([128, TT], F32, tag="y")
                    yw1 = ayp.tile([128, TT], F32, tag="y")
                    yws = {0: yw0, 1: yw1}
                    qcols = [qT_sb[:, hl, 2 * b + qt, :] for qt in range(2)]
                    pws = {}
                    for kt in range(NKT):
                        k0 = kt * 128
                        for qt in range(2):
                            sw = ascp.tile([128, TT], F32, tag="sc")
                            nc.tensor.matmul(
                                sw[:], pk4[:, hl, k0 : k0 + 128], qcols[qt],
                                start=True, stop=True,
                            )
                            pw = pt2.tile([128, TT], BF16, tag="pt2")
                            if kt == NKT - 1:
                                nc.scalar.activation(
                                    pw[:], sw[:], AF.Exp, bias=tailb[:], scale=SCALE
                                )
                            else:
                                nc.scalar.activation(pw[:], sw[:], AF.Exp, scale=SCALE)
                            pws[(qt, kt)] = pw
                        if kt >= 1:
                            for qt in range(2):
                                pwp_ = pws[(qt, kt - 1)]
                                nc.tensor.matmul(
                                    yws[qt][:], pv4[:, kt - 1, :], pwp_[:],
                                    start=(kt - 1 == 0), stop=False,
                                )
                                if kt % 2 == 0:
                                    # sum the pt pair on the DVE; one den
                                    # matmul per TWO key tiles
                                    pa_, pb_ = pws.pop((qt, kt - 2)), pws.pop((qt, kt - 1))
                                    psum2 = pt2.tile([128, TT], BF16, tag="ptsum")
                                    nc.vector.tensor_add(psum2[:], pa_[:], pb_[:])
                                    nc.tensor.matmul(
                                        dt[32 * qt : 32 * qt + 1, :], ones_bf[:],
                                        psum2[:],
                                        start=(kt == 2), stop=False,
                                        skip_group_check=True,
                                    )
                    for qt in range(2):
                        pwp_ = pws[(qt, NKT - 1)]
                        nc.tensor.matmul(
                            yws[qt][:], pv4[:, NKT - 1, :], pwp_[:], start=False, stop=True
                        )
                        pa_, pb_ = pws.pop((qt, NKT - 2)), pws.pop((qt, NKT - 1))
                        psum2 = pt2.tile([128, TT], BF16, tag="ptsum")
                        nc.vector.tensor_add(psum2[:], pa_[:], pb_[:])
                        nc.tensor.matmul(
                            dt[32 * qt : 32 * qt + 1, :], ones_bf[:], psum2[:],
                            start=False, stop=True, skip_group_check=True,
                        )

                    # one reciprocal for both denominators
                    rw = ap.tile([97, TT], F32, tag="rc")
                    rw_bf = ap.tile([97, TT], BF16, tag="rcbf")
                    nc.vector.reciprocal(rw[0:33, :], dt[0:33, :])
                    nc.vector.tensor_copy(rw_bf[0:33, :], rw[0:33, :])

                    # filler PE work hides the recip+cast latency
                    if b == 0:
                        want = ((hl + 1) * len(a2f)) // HPC
                        mid_stop = min(max(a2n + 2, want - 1), len(a2f))
                        while a2n < mid_stop:
                            a2f[a2n]()
                            a2n += 1

                    for qt in range(2):
                        oi = (b * HPC + hl) * 2 + qt
                        dr = 32 * qt
                        rep = ascp.tile([128, TT], F32, tag="sc")
                        nc.tensor.matmul(
                            rep[:], ones128[dr : dr + 1, :], rw_bf[dr : dr + 1, :],
                            start=True, stop=True,
                        )
                        rep_sb = ap.tile([128, TT], F32, tag="repsb")
                        nc.vector.tensor_copy(rep_sb[:], rep[:])
                        tmp = ap.tile([128, TT], F32, tag="tmp")
                        nc.vector.tensor_tensor(tmp[:], yws[qt][:], rep_sb[:], ALU.mult)
                        yfin = ap.tile([128, TT], BF16, tag="yfin")
                        nc.vector.scalar_tensor_tensor(
                            yfin[:], tmp[:], pg, o_store[:, oi, :], ALU.mult, ALU.add
                        )
                        # stage into this batch's a2a bounce: j = tok/128
                        for c4 in range(4):
                            nc.sync.dma_start(
                                a2a_ins[b][qt * 4 + c4, :, hl, :],
                                yfin[:, c4 * 128 : (c4 + 1) * 128],
                            )
                    if b == 0:
                        while a2n < want:
                            a2f[a2n]()
                            a2n += 1
                    elif hl == 0:
                        # deferred past the b1-hl0 input DMAs: these wait on
                        # collective b0 and would head-of-line-block the
                        # Sync queue if emitted right after the collective
                        yT_load(0)
                        w_fetch(0)
                # batch b fully staged: launch its AllToAll (overlaps the
                # next batch's attention / c_proj chunks)
                nc.gpsimd.collective_compute(
                    "AllToAll",
                    ALU.bypass,
                    replica_groups=[list(range(NCORES))],
                    ins=[a2a_ins[b][:].opt()],
                    outs=[a2a_outs[b][:].opt()],
                )


            # ---------------- P tail: each cproj column block streams
            # ONCE; batch-1 chunks lag one block behind batch-0 so the
            # first two batch-0 chunks cover the second collective + yT1
            # latency, and each weight is fully consumed before its ring
            # slots are refetched.
            w_fetch(1)
            yT_load(1)
            for n in range(NP):
                p_chunk(n, 0, pop=False)
                p_chunk(n, 1)
                if n + 2 < NP:
                    w_fetch(n + 2)
            pps.close()

        mid.close()

    _split_multi_waits(nc)
    return nc


def prepare_inputs(inputs):
    """Host-side slicing / casting / transposition. Returns in_maps (one
    dict per core)."""
    f32 = np.float32
    x = np.asarray(inputs["x"], f32)
    audio = np.asarray(inputs["audio_features"], f32)
    rope_cos = np.asarray(inputs["rope_cos"], f32)
    rope_sin = np.asarray(inputs["rope_sin"], f32)
    pad_k = np.asarray(inputs["pad_base_k"], f32)
    pad_v = np.asarray(inputs["pad_base_v"], f32)
    c_attn = np.asarray(inputs["c_attn_w"], f32)
    c_proj = np.asarray(inputs["c_proj_w"], f32)
    adapter_wte = np.asarray(inputs["adapter_wte"], f32)
    rms_gate = np.asarray(inputs["rms_gate_w"], f32)
    rms_key = np.asarray(inputs["rms_key_w"], f32)
    rms_val = np.asarray(inputs["rms_value_w"], f32)
    p_down = np.asarray(inputs["proj_down"], f32)
    p_up = np.asarray(inputs["proj_up"], f32)
    wh_k = np.asarray(inputs["whisper_key_w"], f32)
    wh_v = np.asarray(inputs["whisper_value_w"], f32)
    wh_vb = np.asarray(inputs["whisper_value_b"], f32)

    assert np.array_equal(
        np.asarray(inputs["proj_q128"], f32), np.eye(HS, dtype=f32)
    ) and np.array_equal(
        np.asarray(inputs["proj_q32"], f32), np.eye(NH, dtype=f32)
    ), "general q-reprojection path not implemented"
    mask = np.asarray(inputs["mask"])
    assert mask.shape == (1, 1, T, T)
    assert np.array_equal(
        mask[0, 0], np.tril(np.ones((T, T), dtype=bool))
    ), "only causal mask supported"

    # pre-tiled [tt, p, ko, t]: per-partition-contiguous 32KB DMA slabs
    xT = np.ascontiguousarray(
        x.reshape(NTT, TT, KO, 128).transpose(0, 3, 2, 1).astype(NBF)
    )

    # adapter k/v on host (tiny)
    ms = np.mean(adapter_wte * adapter_wte, axis=-1, keepdims=True)
    prefix = adapter_wte / np.sqrt(ms + EPS) * rms_gate
    aqkv = prefix @ c_attn
    ak = aqkv[:, C : 2 * C].reshape(A_LEN, NH, HS)
    av = aqkv[:, 2 * C :].reshape(A_LEN, NH, HS)

    cosT = np.ascontiguousarray(rope_cos.T)
    sinT = np.ascontiguousarray(rope_sin.T)

    # shared [128,128] triangular mask for the diag-boundary column block
    kk = np.arange(128)[:, None]
    jj = np.arange(128)[None, :]
    masks = np.where(jj >= kk, 0.0, NEG).astype(f32)

    wkey_t = np.ascontiguousarray(
        wh_k.astype(NBF).reshape(NOT, 128, NOT, 128).transpose(2, 1, 0, 3)
    )
    wval_t = np.ascontiguousarray(
        wh_v.astype(NBF).reshape(NOT, 128, NOT, 128).transpose(2, 1, 0, 3)
    )
    vb_t = np.ascontiguousarray(wh_vb.reshape(NOT, 128).T)
    rmsk_t = np.ascontiguousarray(rms_key.reshape(NOT, 128).T)
    rmsv_t = np.ascontiguousarray(rms_val.reshape(NOT, 128).T)
    padkT_perm = np.ascontiguousarray(pad_k.transpose(0, 2, 1)[:, PERM, :])
    cproj_b = np.ascontiguousarray(
        c_proj.astype(NBF).reshape(KO, 128, C // TT, TT).transpose(2, 0, 1, 3)
    )
    aT_full = np.ascontiguousarray(audio.reshape(B * AT, AD).T)  # [1280, 3000]
    # pupk col (u, i) = proj_up[:, 64u + PERM64[i]]
    pupk_all = np.empty((DD, 20 * WHD), f32)
    for u in range(20):
        pupk_all[:, u * WHD : (u + 1) * WHD] = p_up[:, u * WHD + PERM64]

    # pad_v key-tiled [B, 128, 12, 64]: cols 64:128 (non-whisper dims) and
    # cols 0:64 (whisper dims, used as additive base on non-whisper cores)
    pv_pad = np.zeros((B, AT2, HS), f32)
    pv_pad[:, :AT, :] = pad_v
    pv_tiles = pv_pad.reshape(B, NKT, 128, HS).transpose(0, 2, 1, 3)
    padvT_hi = np.ascontiguousarray(pv_tiles[:, :, :, WHD:]).astype(NBF)
    padvT_lo = np.ascontiguousarray(pv_tiles[:, :, :, :WHD]).astype(NBF)
    padvT_lo_zero = np.zeros_like(padvT_lo)

    in_maps = []
    for c in range(NCORES):
        heads = range(HPC * c, HPC * c + HPC)
        wq_c = np.empty((C, HPC * HS), f32)
        wk_c = np.empty((C, HPC * HS), f32)
        wv_c = np.empty((C, HPC * HS), f32)
        akT_c = np.empty((HPC, HS, A_LEN), f32)
        av_c = np.empty((HPC, A_LEN, HS), f32)
        for hl, h in enumerate(heads):
            wq_c[:, hl * HS : (hl + 1) * HS] = c_attn[:, h * HS + PERM]
            wk_c[:, hl * HS : (hl + 1) * HS] = c_attn[:, C + h * HS + PERM]
            wv_c[:, hl * HS : (hl + 1) * HS] = (
                c_attn[:, 2 * C + h * HS : 2 * C + (h + 1) * HS]
            )
            akT_c[hl] = ak[:, h, PERM].T
            av_c[hl] = av[:, h, :]

        wk_core = c * HPC + HPC - 1 < NWH  # all 4 heads whisper-backed
        padkT_c = np.zeros((B, HS, AT2), f32)
        padkT_c[:, :, :AT] = padkT_perm
        if wk_core:
            aT_c = np.empty((AD, B * 300), f32)
            for b in range(B):
                aT_c[:, b * 300 : (b + 1) * 300] = aT_full[
                    :, b * AT + 300 * c : b * AT + 300 * c + 300
                ]
            pupk_c, pupv_c = pupk_all, p_up
            padkT_c[:, 0:32, :] = 0.0
            padkT_c[:, 64:96, :] = 0.0
            padv0_c = padvT_lo_zero
        else:
            aT_c = np.zeros((AD, B * 300), f32)
            pupk_c = np.zeros((DD, 20 * WHD), f32)
            pupv_c = np.zeros((DD, AD), f32)
            padv0_c = padvT_lo

        wq_t = np.ascontiguousarray(
            wq_c.astype(NBF).reshape(KO, 128, HPC, HS).transpose(2, 1, 0, 3)
        )
        wk_t = np.ascontiguousarray(
            wk_c.astype(NBF).reshape(KO, 128, HPC, HS).transpose(2, 1, 0, 3)
        )
        wv_t = np.ascontiguousarray(
            wv_c.astype(NBF).reshape(KO, 128, HPC * HS).transpose(1, 0, 2)
        )
        in_maps.append(
            dict(
                xT=xT,
                wq=wq_t, wk=wk_t, wv=wv_t,
                cosT=cosT, sinT=sinT, masks=masks,
                akT=akT_c.astype(NBF), avd=av_c.astype(NBF),
                aT=aT_c.astype(NBF),
                wkey=wkey_t, wval=wval_t,
                vbias=vb_t, rmsk=rmsk_t, rmsv=rmsv_t,
                pdown=np.ascontiguousarray(
                    np.stack([p_down * rms_key[:, None], p_down * rms_val[:, None]])
                ).astype(NBF),
                pupk=pupk_c.astype(NBF), pupv=pupv_c.astype(NBF),
                padkT=padkT_c.astype(NBF),
                padvT=padvT_hi, padv0=padv0_c,
                cproj=cproj_b,
            )
        )
    return in_maps


def get_program(inputs):
    gf = float(np.asarray(inputs["gating_factor"], np.float32))
    pg = float(np.asarray(inputs["proj_gating"], np.float32))
    key = (gf, pg)
    if key not in _PROG_CACHE:
        _PROG_CACHE[key] = build_program(gf, pg)
    return _PROG_CACHE[key]


def kernel(**inputs) -> np.ndarray:
    nc = get_program(inputs)
    in_maps = prepare_inputs(inputs)
    res = run_bass_kernel_spmd(nc, in_maps, core_ids=list(range(NCORES)))
    # core j rows 0:128 = batch-0 tokens [128j, 128j+128); rows 128:256 = batch 1
    full = np.empty((B, T, C), np.float32)
    for c in range(NCORES):
        r = res.results[c]["out"]
        full[0, 128 * c : 128 * (c + 1)] = r[0:128]
        full[1, 128 * c : 128 * (c + 1)] = r[128:256]
    return full

